# revision 1
# baseline (speedup 1.0000x reference)
"""Trainium2 Bass kernel for a dense transformer decoder block.

Sharding: pure data-parallel over 8 cores. Core c=(b*4+j) handles batch b and
query blocks {4i+j : i=0..3} (128 tokens each, interleaved for causal balance).
v1: every core computes K/V for the full 2048-token batch (no collectives).

All on-device activations are kept TRANSPOSED ([emb, tokens]) so every matmul
has its contraction dim on partitions and nothing ever needs an on-device
transpose; the host pre-transposes inputs and post-transposes outputs.

Status: HW-verified rms rel err 1.711e-4; cost-model makespan 654.7us/core
(TimelineSim; NTFF profiling unavailable in this container).

Ranked remaining levers (from per-engine timeline analysis):
1. AllGather K/V (~150us): replace the duplicated full-batch LN+K/V
   projection (~230us engine work/core) with per-core 512-token K/V + a
   4-rank bf16 AllGather (~2MB/rank, ~20-40us, replica_groups
   [[0..3],[4..7]], Shared-addr DRAM bounce). Attention phase is unchanged.
2. Batched exp (~25-45us): 360 ACT instructions avg 498ns (~180ns fixed
   dispatch each). Group the 2-4 same-q-range key-blocks per (pair, head)
   into one multi-bank PSUM scores tile so one Activation covers them.
   Watch PSUM budget: scores 4-bank tiles + 2x2 attnV accumulators = 8.
3. LN prologue overlap (~30-60us): PE idles ~60% for the first 100us;
   interleave per-512-token-chunk LN with that chunk's K/V projections.
Sim-rejected: psum pools bufs=4 (+20us), gpsimd mask-mul (+2us).
HW gotchas hit: fp32r operands must be produced-as-f32r; fp32r matmuls
carry max 1 inline wait (Bacc.compile splits); partition_broadcast corrupts
at nonzero out base partition; DVE reads max 1 PSUM operand; only gpsimd
DMAs cast dtypes; DMA transpose 4B capped at 64 partitions.
"""

import numpy as np

import concourse.bass as bass
import concourse.bacc as bacc
import concourse.mybir as mybir
import concourse.tile as tile
from concourse.bass_utils import run_bass_kernel_spmd

B, T, C, H, HD, F = 2, 2048, 1024, 16, 64, 4096
EPS = 1e-5
P = 128
CB = C // P          # 8 chunks of emb
FB = F // P          # 32 chunks of ffn dim
TQ = 512             # query tokens per core
NQB = TQ // P        # 4 query blocks per core
TKV = 2048           # kv tokens per core (v1: full batch)
NSB = TKV // P       # 16 key blocks
SCALE = float(C) ** -0.5
NEG = -1e9

F32 = mybir.dt.float32
F32R = mybir.dt.float32r
BF16 = mybir.dt.bfloat16


def _load_col_slice(nc, pool, w_dram, n_rows, col0, ncols, name, bufs=2):
    """Load w[:, col0:col0+ncols] of a [n_rows, *] DRAM matrix into SBUF
    laid out [128, n_rows//128, ncols]."""
    kb = n_rows // P
    t = pool.tile([P, kb, ncols], F32R, name=name, bufs=bufs)
    src = w_dram[:, :].rearrange("(k p) n -> p k n", p=P)[:, :, col0 : col0 + ncols]
    nc.sync.dma_start(out=t, in_=src)
    return t


def build_kernel(dbg=False):
    nc = bacc.Bacc("TRN2", num_devices=8)

    # ---- per-core DRAM I/O ----
    xT_own = nc.dram_tensor("xT_own", [C, TQ], F32R, kind="ExternalInput")
    xT_kv = nc.dram_tensor("xT_kv", [C, TKV], F32R, kind="ExternalInput")
    maskT = nc.dram_tensor("maskT", [P, 4, P], F32, kind="ExternalInput")
    wq = nc.dram_tensor("wq", [C, C], F32R, kind="ExternalInput")
    wk = nc.dram_tensor("wk", [C, C], F32R, kind="ExternalInput")
    wv = nc.dram_tensor("wv", [C, C], F32R, kind="ExternalInput")
    wo = nc.dram_tensor("wo", [C, C], F32R, kind="ExternalInput")
    w1 = nc.dram_tensor("w1", [C, F], F32R, kind="ExternalInput")
    w2 = nc.dram_tensor("w2", [F, C], F32R, kind="ExternalInput")
    gb = nc.dram_tensor("gb", [6, C], F32R, kind="ExternalInput")  # g1,b1,g2,b2,bo,bf2
    bf1 = nc.dram_tensor("bf1", [F], F32, kind="ExternalInput")
    ones_in = nc.dram_tensor("ones_in", [1, TQ], F32R, kind="ExternalInput")
    outT = nc.dram_tensor("outT", [C, TQ], F32, kind="ExternalOutput")
    if dbg:
        d_hown = nc.dram_tensor("d_hown", [P, CB, TQ], F32, kind="ExternalOutput")
        d_hkv = nc.dram_tensor("d_hkv", [P, CB, TKV], F32, kind="ExternalOutput")
        d_qT = nc.dram_tensor("d_qT", [P, CB, TQ], F32, kind="ExternalOutput")
        d_kT = nc.dram_tensor("d_kT", [P, CB, TKV], F32, kind="ExternalOutput")
        d_vaug = nc.dram_tensor("d_vaug", [P, NSB, H, HD + 1], F32,
                                kind="ExternalOutput")
        d_attnT = nc.dram_tensor("d_attnT", [P, CB, TQ], F32, kind="ExternalOutput")
        d_zT = nc.dram_tensor("d_zT", [P, CB, TQ], F32, kind="ExternalOutput")
        d_h2T = nc.dram_tensor("d_h2T", [P, CB, TQ], F32, kind="ExternalOutput")
        d_probs = nc.dram_tensor("d_probs", [P, 2, NSB, TQ], F32,
                                 kind="ExternalOutput")
        d_bc = nc.dram_tensor("d_bc", [P, TQ], F32, kind="ExternalOutput")
        d_den = nc.dram_tensor("d_den", [2, TQ], F32, kind="ExternalOutput")
        d_raw = nc.dram_tensor("d_raw", [P, TQ], F32, kind="ExternalOutput")

    import contextlib

    with tile.TileContext(nc) as tc, contextlib.ExitStack() as ctx:
        singles = ctx.enter_context(tc.tile_pool(name="singles", bufs=1))

        # small constants
        ones_col = singles.tile([P, 1], F32R)
        nc.sync.dma_start(out=ones_col, in_=ones_in[:, 0:1].to_broadcast([P, 1]))
        ones_row = singles.tile([1, TQ], F32R)
        nc.sync.dma_start(out=ones_row, in_=ones_in[:, :])
        eps_t = singles.tile([1, 1], F32)
        nc.vector.memset(eps_t, EPS)

        # g rows on partition 0 (PE-broadcast lhsT); biases as per-partition scalars
        g_rows = singles.tile([1, 2, C], F32R)
        nc.sync.dma_start(out=g_rows[:, 0, :], in_=gb[None, 0, :])
        nc.sync.dma_start(out=g_rows[:, 1, :], in_=gb[None, 2, :])
        g1_row = g_rows[:, 0, :]
        g2_row = g_rows[:, 1, :]
        b1_pc = singles.tile([P, CB], F32)
        nc.sync.dma_start(out=b1_pc, in_=gb[1, :].rearrange("(k p) -> p k", p=P).bitcast(F32))
        b2_pc = singles.tile([P, CB], F32)
        nc.sync.dma_start(out=b2_pc, in_=gb[3, :].rearrange("(k p) -> p k", p=P).bitcast(F32))
        bo_pc = singles.tile([P, CB], F32)
        nc.sync.dma_start(out=bo_pc, in_=gb[4, :].rearrange("(k p) -> p k", p=P).bitcast(F32))
        bf2_pc = singles.tile([P, CB], F32)
        nc.sync.dma_start(out=bf2_pc, in_=gb[5, :].rearrange("(k p) -> p k", p=P).bitcast(F32))
        bf1_pc = singles.tile([P, FB], F32)
        nc.sync.dma_start(out=bf1_pc, in_=bf1[:].rearrange("(k p) -> p k", p=P))
        mask_sb = singles.tile([P, 4, P], BF16)
        nc.gpsimd.dma_start(out=mask_sb, in_=maskT[:, :, :])

        # ---------------- LayerNorm helper (transposed layout) ----------------
        def ln_T(xp, hp, ntok, g_row, b_pc):
            ntc = ntok // TQ
            with contextlib.ExitStack() as c2:
                lnp = c2.enter_context(tc.tile_pool(name="ln_ps", bufs=2, space="PSUM"))
                lns = c2.enter_context(tc.tile_pool(name="ln_sb", bufs=2))
                lnr = c2.enter_context(tc.tile_pool(name="ln_rows", bufs=1))
                for t0 in range(ntc):
                    sl = slice(t0 * TQ, (t0 + 1) * TQ)
                    m_ps = lnp.tile([1, TQ], F32, name="m_ps")
                    s_ps = lnp.tile([1, TQ], F32, name="s_ps")
                    for cb in range(CB):
                        nc.tensor.matmul(m_ps, ones_col, xp[:, cb, sl],
                                         start=(cb == 0), stop=(cb == CB - 1))
                    for cb in range(CB):
                        sq = lns.tile([P, TQ], F32R, name="sq")
                        nc.scalar.activation(sq, xp[:, cb, sl],
                                             mybir.ActivationFunctionType.Square)
                        nc.tensor.matmul(s_ps, ones_col, sq,
                                         start=(cb == 0), stop=(cb == CB - 1))
                    m_sb = lnr.tile([1, TQ], F32, name="m_sb")
                    nc.scalar.mul(m_sb, m_ps, 1.0 / C)
                    var = lnr.tile([1, TQ], F32, name="var")
                    nc.scalar.mul(var, s_ps, 1.0 / C)
                    msq = lnr.tile([1, TQ], F32, name="msq")
                    nc.vector.tensor_mul(msq, m_sb, m_sb)
                    nc.vector.tensor_sub(var, var, msq)
                    nc.scalar.activation(var, var, mybir.ActivationFunctionType.Sqrt,
                                         bias=eps_t)
                    rstd = lnr.tile([1, TQ], F32R, name="rstd")
                    with nc.allow_low_precision(reason="f32r rounding is fine here"):
                        nc.vector.reciprocal(rstd, var)
                    nm = lnr.tile([1, TQ], F32R, name="nm")
                    nc.vector.tensor_mul(nm, m_sb, rstd)
                    nc.scalar.mul(nm, nm, -1.0)
                    for cb in range(CB):
                        csl = slice(cb * P, (cb + 1) * P)
                        sc_ps = lnp.tile([P, TQ], F32, name="sc_ps")
                        bi_ps = lnp.tile([P, TQ], F32, name="bi_ps")
                        nc.tensor.matmul(sc_ps, g_row[:, csl], rstd,
                                         start=True, stop=True)
                        nc.tensor.matmul(bi_ps, g_row[:, csl], nm,
                                         start=True, stop=True)
                        nc.vector.tensor_mul(hp[:, cb, sl], xp[:, cb, sl], sc_ps)
                        nc.vector.scalar_tensor_tensor(
                            out=hp[:, cb, sl], in0=hp[:, cb, sl],
                            scalar=b_pc[:, cb : cb + 1], in1=bi_ps,
                            op0=mybir.AluOpType.add, op1=mybir.AluOpType.add)

        # --- allocation order = reverse free order (pools are a LIFO stack) ---
        attnT64, _free_attnT = tc.tile([HD, H, TQ], BF16, name="attnT64")
        qT, free_qT = tc.tile([P, CB, TQ], BF16, name="qT")
        kT, free_kT = tc.tile([P, CB, TKV], BF16, name="kT")
        v_aug, free_v = tc.tile([P, NSB, H, HD + 1], BF16, name="v_aug")
        nc.vector.memset(v_aug[:, :, :, HD], 1.0)
        h_kvT_t, free_h_kv = tc.tile([P, CB, TKV], F32R, name="h_kvT")
        h_ownT_t, free_h_own = tc.tile([P, CB, TQ], F32R, name="h_ownT")

        # ---------------- phase 1: LN1 ----------------
        for cb in range(CB):
            nc.sync.dma_start(
                out=h_kvT_t[:, cb, :],
                in_=xT_kv[:, :].rearrange("(k p) t -> p k t", p=P)[:, cb, :])
            nc.sync.dma_start(
                out=h_ownT_t[:, cb, :],
                in_=xT_own[:, :].rearrange("(k p) t -> p k t", p=P)[:, cb, :])
        # both LayerNorms in place (raw x_own is re-loaded later for the residual)
        ln_T(h_ownT_t, h_ownT_t, TQ, g1_row, b1_pc)
        ln_T(h_kvT_t, h_kvT_t, TKV, g1_row, b1_pc)

        if dbg:
            nc.sync.dma_start(out=d_hown[:, :, :], in_=h_ownT_t.bitcast(F32))
            for cb in range(CB):
                nc.sync.dma_start(out=d_hkv[:, cb, :],
                                  in_=h_kvT_t[:, cb, :].bitcast(F32))

        # ---------------- phase 2: Q, V, K projections ----------------
        with contextlib.ExitStack() as p2a:
            wps = p2a.enter_context(tc.tile_pool(name="q_ps", bufs=2, space="PSUM"))
            wcols = p2a.enter_context(tc.tile_pool(name="wcols_q", bufs=3))
            for mb in range(CB):
                wq_c = _load_col_slice(nc, wcols, wq, C, mb * P, P, "wq_c")
                ps = wps.tile([P, TQ], F32, name="ps_q")
                for kb in range(CB):
                    nc.tensor.matmul(ps, wq_c[:, kb, :], h_ownT_t[:, kb, :],
                                     start=(kb == 0), stop=(kb == CB - 1))
                nc.vector.tensor_copy(qT[:, mb, :], ps)
        free_h_own()

        with contextlib.ExitStack() as p2b:
            wps = p2b.enter_context(tc.tile_pool(name="v_ps", bufs=2, space="PSUM"))
            wv_pool = p2b.enter_context(tc.tile_pool(name="wv_pool", bufs=1))
            for nb in range(2):
                wv_sb = wv_pool.tile([P, CB, TQ], F32R, name="wv_half")
                nc.sync.dma_start(
                    out=wv_sb,
                    in_=wv[:, :].rearrange("(k p) n -> p k n", p=P)[
                        :, :, nb * TQ : (nb + 1) * TQ])
                for tb in range(NSB):
                    ps = wps.tile([P, TQ], F32, name="ps_v")
                    for kb in range(CB):
                        nc.tensor.matmul(
                            ps, h_kvT_t[:, kb, tb * P : (tb + 1) * P],
                            wv_sb[:, kb, :],
                            start=(kb == 0), stop=(kb == CB - 1))
                    nc.vector.tensor_copy(
                        v_aug[:, tb, nb * 8 : (nb + 1) * 8, 0:HD],
                        ps.rearrange("p (h d) -> p h d", d=HD))

        with contextlib.ExitStack() as p2c:
            wps = p2c.enter_context(tc.tile_pool(name="k_ps", bufs=2, space="PSUM"))
            wcols = p2c.enter_context(tc.tile_pool(name="wcols_k", bufs=3))
            for mb in range(CB):
                wk_c = _load_col_slice(nc, wcols, wk, C, mb * P, P, "wk_c")
                for t0 in range(TKV // TQ):
                    ps = wps.tile([P, TQ], F32, name="ps_k")
                    sl = slice(t0 * TQ, (t0 + 1) * TQ)
                    for kb in range(CB):
                        nc.tensor.matmul(ps, wk_c[:, kb, :], h_kvT_t[:, kb, sl],
                                         start=(kb == 0), stop=(kb == CB - 1))
                    nc.vector.tensor_copy(kT[:, mb, sl], ps)
        if dbg:
            nc.gpsimd.dma_start(out=d_qT[:, :, :], in_=qT)
            for cb in range(CB):
                nc.gpsimd.dma_start(out=d_kT[:, cb, :], in_=kT[:, cb, :])
            for sb in range(NSB):
                nc.gpsimd.dma_start(out=d_vaug[:, sb, :, :], in_=v_aug[:, sb, :, :])
        free_h_kv()

        # ---------------- phase 3: attention (per head pair) ----------------
        with contextlib.ExitStack() as p3:
            sc_ps_pool = p3.enter_context(
                tc.tile_pool(name="sc_ps", bufs=3, space="PSUM"))
            pair_ps_pool = p3.enter_context(
                tc.tile_pool(name="pair_ps", bufs=2, space="PSUM"))
            bc_pool = p3.enter_context(tc.tile_pool(name="bc", bufs=2))
            probs_pool = p3.enter_context(tc.tile_pool(name="probs", bufs=2))
            rec_pool = p3.enter_context(tc.tile_pool(name="rec", bufs=2))

            for pair in range(H // 2):
                probsT = probs_pool.tile([P, 2, NSB, TQ], BF16, name="probsT")
                ps_h = [pair_ps_pool.tile([HD + 1, TQ], F32, name=f"ps_h{u}")
                        for u in range(2)]
                for sb in range(NSB):
                    q_lo = (sb // 4) * P
                    n = TQ - q_lo
                    d = sb % 4
                    for u in range(2):  # head h = 2*pair+u
                        prow = slice(u * HD, (u + 1) * HD)
                        ps_s = sc_ps_pool.tile([P, TQ], F32, name="ps_s")
                        nc.tensor.matmul(
                            ps_s[:, 0:n],
                            kT[prow, pair, sb * P : (sb + 1) * P],
                            qT[prow, pair, q_lo:TQ],
                            start=True, stop=True)
                        nc.scalar.activation(
                            probsT[:, u, sb, q_lo:TQ], ps_s[:, 0:n],
                            mybir.ActivationFunctionType.Exp, scale=SCALE)
                        nc.vector.tensor_mul(
                            probsT[:, u, sb, q_lo : q_lo + P],
                            probsT[:, u, sb, q_lo : q_lo + P],
                            mask_sb[:, d, :])
                for sb in range(NSB):
                    q_lo = (sb // 4) * P
                    first, last = (sb == 0), (sb == NSB - 1)
                    for u in range(2):
                        h = 2 * pair + u
                        nc.tensor.matmul(
                            ps_h[u][:, q_lo:TQ],
                            v_aug[:, sb, h, :],
                            probsT[:, u, sb, q_lo:TQ],
                            start=first, stop=last)
                bc_sb = bc_pool.tile([HD, 2, TQ], F32, name="bc_sb")
                rec_pair = rec_pool.tile([1, 2, TQ], F32, name="rec_pair")
                for u in range(2):
                    nc.vector.reciprocal(rec_pair[:, u, :],
                                         ps_h[u][HD : HD + 1, :])
                nc.gpsimd.partition_broadcast(bc_sb, rec_pair)
                for u in range(2):
                    nc.vector.tensor_mul(attnT64[:, 2 * pair + u, :],
                                         ps_h[u][0:HD, :], bc_sb[:, u, :])
                if dbg and pair == 0:
                    nc.gpsimd.dma_start(out=d_probs[:, :, :, :], in_=probsT)
        if dbg:
            for h in range(H):
                nc.gpsimd.dma_start(
                    out=d_attnT[(h % 2) * HD : (h % 2) * HD + HD, h // 2, :],
                    in_=attnT64[:, h, :])
        free_v()
        free_kT()
        free_qT()

        # FFN1 weight pool opened early: its first loads overlap wo/LN2
        prefetch = contextlib.ExitStack()
        w1c = prefetch.enter_context(tc.tile_pool(name="w1c", bufs=2))

        # ---------------- phase 4: wo + residual + LN2 ----------------
        zT, _free_zT = tc.tile([P, CB, TQ], F32R, name="zT")
        x_ownT, free_x_own = tc.tile([P, CB, TQ], F32R, name="x_ownT")
        for cb in range(CB):
            nc.sync.dma_start(
                out=x_ownT[:, cb, :],
                in_=xT_own[:, :].rearrange("(k p) t -> p k t", p=P)[:, cb, :])
        with contextlib.ExitStack() as p4:
            ops = p4.enter_context(tc.tile_pool(name="wo_ps", bufs=3, space="PSUM"))
            wcols4 = p4.enter_context(tc.tile_pool(name="wcols4", bufs=3))
            for mb in range(CB):
                wo_c = wcols4.tile([HD, H, P], BF16, name="wo_c", bufs=3)
                nc.gpsimd.dma_start(
                    out=wo_c,
                    in_=wo[:, :].rearrange("(h d) m -> d h m", d=HD)[
                        :, :, mb * P : (mb + 1) * P])
                ps = ops.tile([P, TQ], F32, name="ps_y")
                for h in range(H):
                    nc.tensor.matmul(ps, wo_c[:, h, :], attnT64[:, h, :],
                                     start=(h == 0), stop=(h == H - 1))
                nc.vector.scalar_tensor_tensor(
                    out=zT[:, mb, :], in0=ps, scalar=bo_pc[:, mb : mb + 1],
                    in1=x_ownT[:, mb, :],
                    op0=mybir.AluOpType.add, op1=mybir.AluOpType.add)
        free_x_own()

        # ---------------- phase 4b/5: LN2 + FFN ----------------
        if dbg:
            nc.sync.dma_start(out=d_zT[:, :, :], in_=zT.bitcast(F32))
        aT, free_aT = tc.tile([P, FB, TQ], F32R, name="aT")
        h2T, free_h2T = tc.tile([P, CB, TQ], F32R, name="h2T")
        ln_T(zT, h2T, TQ, g2_row, b2_pc)
        if dbg:
            nc.sync.dma_start(out=d_h2T[:, :, :], in_=h2T.bitcast(F32))

        with contextlib.ExitStack() as p5:
            fps = p5.enter_context(tc.tile_pool(name="ffn_ps", bufs=4, space="PSUM"))
            for fg in range(FB // 2):
                w1_c = _load_col_slice(nc, w1c, w1, C, fg * 2 * P, 2 * P, "w1_c",
                                       bufs=2)
                for fi in range(2):
                    fb = fg * 2 + fi
                    ps = fps.tile([P, TQ], F32, name="ps_a")
                    for kb in range(CB):
                        nc.tensor.matmul(ps,
                                         w1_c[:, kb, fi * P : (fi + 1) * P],
                                         h2T[:, kb, :],
                                         start=(kb == 0), stop=(kb == CB - 1))
                    nc.scalar.activation(aT[:, fb, :], ps,
                                         mybir.ActivationFunctionType.Relu,
                                         bias=bf1_pc[:, fb : fb + 1])
        free_h2T()

        with contextlib.ExitStack() as p6:
            fps2 = p6.enter_context(tc.tile_pool(name="ffn2_ps", bufs=3, space="PSUM"))
            w2c = p6.enter_context(tc.tile_pool(name="w2c", bufs=2))
            outp = p6.enter_context(tc.tile_pool(name="outp", bufs=2))
            for mg in range(CB // 2):
                w2_c = _load_col_slice(nc, w2c, w2, F, mg * 2 * P, 2 * P, "w2_c",
                                       bufs=2)
                for mi in range(2):
                    mb = mg * 2 + mi
                    ps = fps2.tile([P, TQ], F32, name="ps_o")
                    for kb in range(FB):
                        nc.tensor.matmul(ps,
                                         w2_c[:, kb, mi * P : (mi + 1) * P],
                                         aT[:, kb, :],
                                         start=(kb == 0), stop=(kb == FB - 1))
                    o_sb = outp.tile([P, TQ], F32, name="o_sb")
                    nc.vector.scalar_tensor_tensor(
                        out=o_sb, in0=ps, scalar=bf2_pc[:, mb : mb + 1],
                        in1=zT[:, mb, :],
                        op0=mybir.AluOpType.add, op1=mybir.AluOpType.add)
                    nc.sync.dma_start(
                        out=outT[:, :].rearrange("(k p) t -> p k t", p=P)[:, mb, :],
                        in_=o_sb)
        free_aT()
        _free_zT()
        prefetch.close()
        _free_attnT()
    nc.compile()
    return nc


_CACHE = {}


def _get_built(dbg=False):
    key = "nc_dbg" if dbg else "nc"
    if key not in _CACHE:
        _CACHE[key] = build_kernel(dbg=dbg)
    return _CACHE[key]


def _qidx(j):
    """Global token indices (within a batch) of core j's query tokens."""
    return np.concatenate([np.arange((4 * i + j) * P, (4 * i + j + 1) * P)
                           for i in range(NQB)])


def _build_in_maps(x, wq, wk, wv, wo, bo, g1, b1, g2, b2, w1, bf1, w2, bf2):
    x = np.asarray(x, np.float32)
    f = np.float32
    wq_m = np.ascontiguousarray(np.asarray(wq, f).transpose(1, 0, 2).reshape(C, C))
    wk_m = np.ascontiguousarray(np.asarray(wk, f).transpose(1, 0, 2).reshape(C, C))
    wv_m = np.ascontiguousarray(np.asarray(wv, f).transpose(1, 0, 2).reshape(C, C))
    wo_m = np.ascontiguousarray(np.asarray(wo, f))
    w1_m = np.ascontiguousarray(np.asarray(w1, f))
    w2_m = np.ascontiguousarray(np.asarray(w2, f))
    gb = np.ascontiguousarray(np.stack([np.asarray(a, f) for a in
                                        (g1, b1, g2, b2, bo, bf2)]))
    bf1_m = np.ascontiguousarray(np.asarray(bf1, f))

    in_maps = []
    for c in range(8):
        b, j = divmod(c, 4)
        qi = _qidx(j)
        xT_own = np.ascontiguousarray(x[b][qi].T)
        xT_kv = np.ascontiguousarray(x[b].T)
        # maskT[p, d, c] = 0 if (j-d)*128 + c >= p else NEG
        pp = np.arange(P)[:, None, None]
        dd = np.arange(4)[None, :, None]
        cc = np.arange(P)[None, None, :]
        maskT = np.where((j - dd) * P + cc >= pp, 1.0, 0.0).astype(f)
        in_maps.append({
            "xT_own": xT_own, "xT_kv": xT_kv, "maskT": maskT,
            "wq": wq_m, "wk": wk_m, "wv": wv_m, "wo": wo_m,
            "w1": w1_m, "w2": w2_m, "gb": gb, "bf1": bf1_m,
            "ones_in": np.ones((1, TQ), np.float32),
        })

    return in_maps


def _gather(results):
    out = np.empty((B, T, C), np.float32)
    for c in range(8):
        b, j = divmod(c, 4)
        out[b, _qidx(j)] = results[c]["outT"].T
    return out


def kernel(**inputs):
    in_maps = _build_in_maps(**inputs)
    nc = _get_built()
    res = run_bass_kernel_spmd(nc, in_maps, core_ids=list(range(8)))
    return _gather(res.results)


def run_debug(**inputs):
    in_maps = _build_in_maps(**inputs)
    nc = _get_built(dbg=True)
    res = run_bass_kernel_spmd(nc, in_maps, core_ids=list(range(8)))
    return res.results


def run_traced(**inputs):
    """Like kernel() but with NTFF tracing; returns BassKernelResults."""
    in_maps = _build_in_maps(**inputs)
    nc = _get_built()
    return run_bass_kernel_spmd(nc, in_maps, core_ids=list(range(8)), trace=True)



# revision 16
# speedup vs baseline: 1.6779x; 1.6779x over previous
"""Trainium2 Bass kernel for a dense transformer decoder block (v2).

Sharding: pure data-parallel over 8 cores. Core c=(b*4+j) owns batch b and
query blocks {4i+j : i=0..3}. Host PERMUTES each core's 2048 tokens so the
core's own 512 query tokens come first; causality is enforced by per-core
per-key-block boundary masks, so the device program is identical on all
cores (j only affects host-prepared data).

Numerics (rms rel err budget ~2.3e-3 vs 2e-2 gate, verified by host emu):
- x, K/Q in bf16; probs/V/attn in fp8e4 (attention output is ~1.5% of the
  residual stream here: scores carry C**-0.5 scaling so softmax is nearly
  uniform -> fp8 noise in the attention path is negligible downstream).
- Q/K/V and wo projections run fp8e4 with DoubleRow perf mode (2 x 128-deep
  contraction per matmul at 0.5 cycles/row). Weights host-scaled x32; the
  scale folds into the softmax scale (1/1024), cancels in the softmax
  normalize, and is divided out in the z epilogue (1/1024).
- FFN stays bf16 (fp8 there costs ~1e-2 rms - too close to the gate).

Cost-model facts this build exploits (instruction_cost_v2.rs):
- matmul time = out_free_size * pe_cycle * cpr; cpr: bf16/f32r(>=256 free)=1.0,
  fp8 DoubleRow=0.5 (with 256-deep contraction -> 4x bf16 FLOP rate).
- PE p-state ramps with *continuous* busy time (low 1.54ns/row after idle,
  0.83 mid, 0.42 full after 3us) -> LN statistic matmuls are emitted batched
  and interleaved so the PE never sleeps between them.
- Collectives cost 15us fixed + 40GB/s -> no collectives; duplicated
  K/V projection is cheap in fp8-DoubleRow instead.
"""

import contextlib

import numpy as np
import ml_dtypes

import concourse.bass as bass
import concourse.bacc as bacc
import concourse.mybir as mybir
import concourse.tile as tile
from concourse.bass_utils import run_bass_kernel_spmd

B, T, C, H, HD, F = 2, 2048, 1024, 16, 64, 4096
EPS = 1e-5
P = 128
CB = C // P          # 8 emb chunks
KP = CB // 2         # 4 DoubleRow chunk-pairs
FB = F // P          # 32 ffn chunks
TQ = 512             # own query tokens per core
TKV = 2048           # kv tokens (full batch, permuted: own 512 first)
NSB = TKV // P       # 16 key blocks
WS = 32.0            # host weight scale for fp8 (wq/wk/wv/wo)
SCALE2 = float(C) ** -0.5 / (WS * WS)   # folds both x32 into softmax scale
LOG32 = float(np.log(32.0))             # probs8 = 32*exp(score): fp8 range

F32 = mybir.dt.float32
F32R = mybir.dt.float32r
BF16 = mybir.dt.bfloat16
FP8 = mybir.dt.float8e4
DR = mybir.MatmulPerfMode.DoubleRow
ADD = mybir.AluOpType.add
MUL = mybir.AluOpType.mult

# q_lo per key block kb (j-independent under the own-first permutation):
# own blocks kb<4 sit at q block kb; non-own block n=kb-4 needs q blocks
# i >= n//3 (boundary block handled by mask).
QL = [kb * P if kb < 4 else ((kb - 4) // 3) * P for kb in range(NSB)]


def build_kernel():
    nc = bacc.Bacc("TRN2", num_devices=8)

    xT = nc.dram_tensor("xT", [C, TKV], BF16, kind="ExternalInput")
    maskT = nc.dram_tensor("maskT", [P, NSB, P], FP8, kind="ExternalInput")
    wq8 = nc.dram_tensor("wq8", [C, C], FP8, kind="ExternalInput")
    wk8 = nc.dram_tensor("wk8", [C, C], FP8, kind="ExternalInput")
    wv8 = nc.dram_tensor("wv8", [C, C], FP8, kind="ExternalInput")
    wo8 = nc.dram_tensor("wo8", [C, C], FP8, kind="ExternalInput")
    w1 = nc.dram_tensor("w1", [C, F], BF16, kind="ExternalInput")
    w2 = nc.dram_tensor("w2", [F, C], BF16, kind="ExternalInput")
    gb = nc.dram_tensor("gb", [6, C], F32R, kind="ExternalInput")
    bf1 = nc.dram_tensor("bf1", [F], F32, kind="ExternalInput")
    ones_in = nc.dram_tensor("ones_in", [1, P], F32R, kind="ExternalInput")
    outT = nc.dram_tensor("outT", [C, TQ], F32, kind="ExternalOutput")

    with tile.TileContext(nc) as tc, contextlib.ExitStack() as ctx:
        singles = ctx.enter_context(tc.tile_pool(name="singles", bufs=1))

        ones_fr = singles.tile([P, 1], F32R)
        nc.sync.dma_start(out=ones_fr, in_=ones_in[:, 0:1].to_broadcast([P, 1]))
        ones_bf = singles.tile([P, 1], BF16)
        nc.vector.memset(ones_bf, 1.0)
        eps_t = singles.tile([1, 1], F32)
        nc.vector.memset(eps_t, EPS)
        log32_t = singles.tile([P, 1], F32)
        nc.vector.memset(log32_t, LOG32)

        g_rows = singles.tile([1, 2, C], BF16)
        nc.gpsimd.dma_start(out=g_rows[:, 0, :], in_=gb[None, 0, :].bitcast(F32))
        nc.gpsimd.dma_start(out=g_rows[:, 1, :], in_=gb[None, 2, :].bitcast(F32))
        b1_pc = singles.tile([P, CB], F32)
        nc.sync.dma_start(out=b1_pc, in_=gb[1, :].rearrange("(k p) -> p k", p=P).bitcast(F32))
        b2_pc = singles.tile([P, CB], F32)
        nc.sync.dma_start(out=b2_pc, in_=gb[3, :].rearrange("(k p) -> p k", p=P).bitcast(F32))
        bo_pc = singles.tile([P, CB], F32)
        nc.sync.dma_start(out=bo_pc, in_=gb[4, :].rearrange("(k p) -> p k", p=P).bitcast(F32))
        bf2_pc = singles.tile([P, CB], F32)
        nc.sync.dma_start(out=bf2_pc, in_=gb[5, :].rearrange("(k p) -> p k", p=P).bitcast(F32))
        bf1_pc = singles.tile([P, FB], F32)
        nc.sync.dma_start(out=bf1_pc, in_=bf1[:].rearrange("(k p) -> p k", p=P))
        mask_sb = singles.tile([P, NSB, P], FP8)
        nc.sync.dma_start(out=mask_sb, in_=maskT[:, :, :])

        # --- persistent activation tiles (alloc order = reverse free order) ---
        xpbo, _f_xpbo = tc.tile([P, CB, TQ], BF16, name="xpbo")  # x + bo (own)
        qT, _f_qT = tc.tile([P, CB, TQ], FP8, name="qT")         # 32*q
        kT, _f_kT = tc.tile([P, CB, TKV], FP8, name="kT")        # 32*k
        v_aug, _f_v = tc.tile([P, NSB, H, HD + 1], FP8, name="v_aug")
        nc.vector.memset(v_aug[:, :, :, HD], 1.0)
        _att = [tc.tile([HD, H // 2, TQ], FP8, name=f"attn8u{u}")
                for u in range(2)]
        attn8u = [t for t, _ in _att]
        attn128, _f_a128 = tc.tile([P, H // 2, TQ], FP8, name="attn128")
        h8, free_h8 = tc.tile([P, CB, TKV], FP8, name="h8")
        w8pool = contextlib.ExitStack()
        w8p = w8pool.enter_context(tc.tile_pool(name="w8p", bufs=1))
        wq_sb = w8p.tile([P, CB, C], FP8, name="wq_sb")
        nc.sync.dma_start(out=wq_sb, in_=wq8[:, :].rearrange("(k p) n -> p k n", p=P))
        wk_sb = w8p.tile([P, CB, C], FP8, name="wk_sb")
        nc.sync.dma_start(out=wk_sb, in_=wk8[:, :].rearrange("(k p) n -> p k n", p=P))
        wv_sb = w8p.tile([P, CB, C], FP8, name="wv_sb")
        nc.sync.dma_start(out=wv_sb, in_=wv8[:, :].rearrange("(k p) n -> p k n", p=P))
        x_sb, free_x = tc.tile([P, CB, TKV], BF16, name="x_sb")

        for cb in range(CB):
            nc.sync.dma_start(
                out=x_sb[:, cb, :],
                in_=xT[:, :].rearrange("(k p) t -> p k t", p=P)[:, cb, :])

        # xpbo = x + bo on own columns (feeds the z residual later)
        for cb in range(CB):
            eng = nc.vector if cb % 2 == 0 else nc.gpsimd
            eng.tensor_scalar_add(xpbo[:, cb, :], x_sb[:, cb, 0:TQ],
                                  bo_pc[:, cb : cb + 1])

        # ================= LN1 over all TKV tokens -> h8 (fp8) =================
        NCH = TKV // TQ  # 4 chunks of 512
        with contextlib.ExitStack() as lnc:
            stat_ps = lnc.enter_context(tc.tile_pool(name="ln_stat", bufs=2, space="PSUM"))
            ap_ps = lnc.enter_context(tc.tile_pool(name="ln_ap", bufs=1, space="PSUM"))
            lns = lnc.enter_context(tc.tile_pool(name="ln_sq", bufs=3))
            lnr = lnc.enter_context(tc.tile_pool(name="ln_rows", bufs=2))
            tmp_p = lnc.enter_context(tc.tile_pool(name="ln_tmp", bufs=2))

            m_tiles, s_tiles, r_tiles = {}, {}, {}

            def ln_means(t0, src, ntok):
                sl = slice(t0 * ntok, (t0 + 1) * ntok)
                m_ps = stat_ps.tile([1, ntok], F32, name="m_ps")
                for cb in range(CB):
                    nc.tensor.matmul(m_ps, ones_bf, src[:, cb, sl],
                                     start=(cb == 0), stop=(cb == CB - 1))
                m_tiles[t0] = m_ps

            def ln_sqs(t0, src, ntok, sq_dt=BF16, ones=None):
                sl = slice(t0 * ntok, (t0 + 1) * ntok)
                s_ps = stat_ps.tile([1, ntok], F32, name="s_ps")
                for cb in range(CB):
                    sq = lns.tile([P, ntok], F32R, name="sq")
                    nc.scalar.activation(sq, src[:, cb, sl],
                                         mybir.ActivationFunctionType.Square)
                    nc.tensor.matmul(s_ps, ones_fr, sq,
                                     start=(cb == 0), stop=(cb == CB - 1))
                s_tiles[t0] = s_ps

            def ln_stats(t0, ntok):
                m_ps, s_ps = m_tiles.pop(t0), s_tiles.pop(t0)
                m_sb = lnr.tile([1, ntok], F32, name="m_sb")
                nc.scalar.mul(m_sb, m_ps, 1.0 / C)
                var = lnr.tile([1, ntok], F32, name="var")
                nc.scalar.mul(var, s_ps, 1.0 / C)
                msq = lnr.tile([1, ntok], F32, name="msq")
                nc.vector.tensor_mul(msq, m_sb, m_sb)
                nc.vector.tensor_sub(var, var, msq)
                nc.scalar.activation(var, var, mybir.ActivationFunctionType.Sqrt,
                                     bias=eps_t)
                rstd = lnr.tile([1, ntok], BF16, name="rstd")
                with nc.allow_low_precision(reason="f32r rstd"):
                    nc.vector.reciprocal(rstd, var)
                nm = lnr.tile([1, ntok], BF16, name="nm")
                nc.vector.tensor_mul(nm, m_sb, rstd)
                nc.scalar.mul(nm, nm, -1.0)
                r_tiles[t0] = (rstd, nm)

            def ln_apply(t0, src, dst, ntok, g_row, b_pc):
                sl = slice(t0 * ntok, (t0 + 1) * ntok)
                rstd, nm = r_tiles.pop(t0)
                sc_ps = ap_ps.tile([P, ntok], F32, name="sc_ps")
                bi_ps = ap_ps.tile([P, ntok], F32, name="bi_ps")
                for cb in range(CB):
                    csl = slice(cb * P, (cb + 1) * P)
                    nc.tensor.matmul(sc_ps, g_row[:, csl], rstd, start=True, stop=True)
                    nc.tensor.matmul(bi_ps, g_row[:, csl], nm, start=True, stop=True)
                    tmp = tmp_p.tile([P, ntok], F32R, name=f"tmp{cb % 2}")
                    nc.vector.tensor_mul(tmp, src[:, cb, sl], sc_ps)
                    nc.vector.scalar_tensor_tensor(
                        out=dst[:, cb, sl], in0=tmp,
                        scalar=b_pc[:, cb : cb + 1], in1=bi_ps,
                        op0=ADD, op1=ADD)

            g1_row = g_rows[:, 0, :]
            # interleave so the PE pipeline never drains
            ln_means(0, x_sb, TQ)
            ln_sqs(0, x_sb, TQ)
            ln_means(1, x_sb, TQ)
            ln_stats(0, TQ)
            ln_sqs(1, x_sb, TQ)
            ln_apply(0, x_sb, h8, TQ, g1_row, b1_pc)
            ln_means(2, x_sb, TQ)
            ln_stats(1, TQ)
            ln_sqs(2, x_sb, TQ)
            ln_apply(1, x_sb, h8, TQ, g1_row, b1_pc)
            ln_means(3, x_sb, TQ)
            ln_stats(2, TQ)
            ln_sqs(3, x_sb, TQ)
            ln_apply(2, x_sb, h8, TQ, g1_row, b1_pc)
            ln_stats(3, TQ)
            ln_apply(3, x_sb, h8, TQ, g1_row, b1_pc)

            # ---- Q projection (own 512 tokens), fp8 DoubleRow ----
            with contextlib.ExitStack() as pq:
                qps = pq.enter_context(tc.tile_pool(name="q_ps", bufs=2, space="PSUM"))
                for mb in range(CB):
                    ps = qps.tile([P, 2, 256], F32, name="ps_q")
                    for f in range(2):
                        fsl = slice(f * 256, (f + 1) * 256)
                        for kp in range(KP):
                            ksl = slice(2 * kp, 2 * kp + 2)
                            nc.tensor.matmul(
                                ps[:, f, :], wq_sb[:, ksl, mb * P : (mb + 1) * P],
                                h8[:, ksl, fsl],
                                start=(kp == 0), stop=(kp == KP - 1), perf_mode=DR)
                    nc.scalar.mul(qT[:, mb, :], ps.rearrange("p f n -> p (f n)"), 1.0)

        # ---- K/V projections (full 2048 tokens), fp8 DoubleRow ----
        with contextlib.ExitStack() as pkv:
            kps = pkv.enter_context(tc.tile_pool(name="k_ps", bufs=6, space="PSUM"))
            vps = pkv.enter_context(tc.tile_pool(name="v_ps", bufs=2, space="PSUM"))
            for mb in range(CB):
                for tf in range(TKV // 256):
                    fsl = slice(tf * 256, (tf + 1) * 256)
                    ps = kps.tile([P, 256], F32, name="ps_k")
                    for kp in range(KP):
                        ksl = slice(2 * kp, 2 * kp + 2)
                        nc.tensor.matmul(
                            ps, wk_sb[:, ksl, mb * P : (mb + 1) * P],
                            h8[:, ksl, fsl],
                            start=(kp == 0), stop=(kp == KP - 1), perf_mode=DR)
                    eng = nc.scalar if tf % 2 == 0 else None
                    if eng is nc.scalar:
                        nc.scalar.mul(kT[:, mb, fsl], ps, 1.0)
                    else:
                        nc.vector.tensor_copy(kT[:, mb, fsl], ps)
            for tb in range(NSB):
                for hh in range(2):  # 8-head halves of the hd axis
                    ps = vps.tile([P, 2, 256], F32, name="ps_v")
                    for f in range(2):
                        fsl = slice(hh * 512 + f * 256, hh * 512 + (f + 1) * 256)
                        for kp in range(KP):
                            ksl = slice(2 * kp, 2 * kp + 2)
                            nc.tensor.matmul(
                                ps[:, f, :],
                                h8[:, ksl, tb * P : (tb + 1) * P],
                                wv_sb[:, ksl, fsl],
                                start=(kp == 0), stop=(kp == KP - 1), perf_mode=DR)
                    if tb % 2 == 0:
                        nc.vector.tensor_copy(
                            v_aug[:, tb, hh * 8 : (hh + 1) * 8, 0:HD],
                            ps.rearrange("p f (h d) -> p (f h) d", d=HD))
                    else:
                        nc.scalar.mul(
                            v_aug[:, tb, hh * 8 : (hh + 1) * 8, 0:HD],
                            ps.rearrange("p f (h d) -> p (f h) d", d=HD), 1.0)
        free_x()
        w8pool.close()
        free_h8()

        zT, _f_zT = tc.tile([P, CB, TQ], F32R, name="zT")
        h2T, _f_h2 = tc.tile([P, CB, TQ], BF16, name="h2T")
        aT, _f_aT = tc.tile([P, FB, TQ], BF16, name="aT")

        # prefetch wo and first FFN1 weights during attention
        # (w1c opens first: pools are a LIFO stack and wo_p closes earlier)
        w1pool = contextlib.ExitStack()
        w1c = w1pool.enter_context(tc.tile_pool(name="w1c", bufs=2))
        wo_pool = contextlib.ExitStack()
        wo_sb = wo_pool.enter_context(tc.tile_pool(name="wo_p", bufs=1)).tile(
            [P, CB, C], FP8, name="wo_sb")
        nc.sync.dma_start(out=wo_sb, in_=wo8[:, :].rearrange("(k p) n -> p k n", p=P))

        # ================= attention (per head pair) =================
        with contextlib.ExitStack() as p3:
            sc_ps_pool = p3.enter_context(tc.tile_pool(name="sc_ps", bufs=2, space="PSUM"))
            pair_ps_pool = p3.enter_context(tc.tile_pool(name="pair_ps", bufs=2, space="PSUM"))
            bc_pool = p3.enter_context(tc.tile_pool(name="bc", bufs=2))
            probs_pool = p3.enter_context(tc.tile_pool(name="probs", bufs=2))
            rec_pool = p3.enter_context(tc.tile_pool(name="rec", bufs=2))

            for pair in range(H // 2):
                probs8 = probs_pool.tile([P, 2, NSB, TQ], FP8, name="probs8")
                ps_h = [pair_ps_pool.tile([HD + 1, TQ], F32, name=f"ps_h{u}")
                        for u in range(2)]
                for g in range(NSB // 2):
                    for u in range(2):
                        prow = slice(u * HD, (u + 1) * HD)
                        ps2 = sc_ps_pool.tile([P, 2, TQ], F32, name="ps_s")
                        for s in range(2):
                            sb = 2 * g + s
                            q0 = QL[sb]
                            nc.tensor.matmul(
                                ps2[:, s, q0:TQ],
                                kT[prow, pair, sb * P : (sb + 1) * P],
                                qT[prow, pair, q0:TQ],
                                start=True, stop=True)
                        qmin = QL[2 * g]
                        nc.scalar.activation(
                            probs8[:, u, 2 * g : 2 * g + 2, qmin:TQ],
                            ps2[:, :, qmin:TQ],
                            mybir.ActivationFunctionType.Exp,
                            scale=SCALE2, bias=log32_t)
                        for s in range(2):
                            sb = 2 * g + s
                            q0 = QL[sb]
                            nc.vector.tensor_mul(
                                probs8[:, u, sb, q0 : q0 + P],
                                probs8[:, u, sb, q0 : q0 + P],
                                mask_sb[:, sb, :])
                for sb in range(NSB):
                    first, last = (sb == 0), (sb == NSB - 1)
                    for u in range(2):
                        nc.tensor.matmul(
                            ps_h[u][:, QL[sb]:TQ],
                            v_aug[:, sb, 2 * pair + u, :],
                            probs8[:, u, sb, QL[sb]:TQ],
                            start=first, stop=last)
                rec_pair = rec_pool.tile([1, 2, TQ], BF16, name="rec_pair")
                for u in range(2):
                    with nc.allow_low_precision(reason="softmax denom"):
                        nc.vector.reciprocal(rec_pair[:, u, :],
                                             ps_h[u][HD : HD + 1, :])
                bc_sb = bc_pool.tile([HD, 2, TQ], BF16, name="bc_sb")
                nc.gpsimd.partition_broadcast(bc_sb, rec_pair)
                for u in range(2):
                    nc.vector.tensor_mul(attn8u[u][:, pair, :],
                                         ps_h[u][0:HD, :], bc_sb[:, u, :])

        # stack head pairs onto 128 partitions (SBUF->SBUF DMA)
        nc.sync.dma_start(out=attn128[0:HD, :, :], in_=attn8u[0])
        nc.sync.dma_start(out=attn128[HD:P, :, :], in_=attn8u[1])

        # ================= wo (fp8 DoubleRow) + residual -> zT =================
        with contextlib.ExitStack() as p4:
            ops = p4.enter_context(tc.tile_pool(name="wo_ps", bufs=2, space="PSUM"))
            for mb in range(CB):
                ps = ops.tile([P, 2, 256], F32, name="ps_z")
                for f in range(2):
                    fsl = slice(f * 256, (f + 1) * 256)
                    for kp in range(KP):
                        ksl = slice(2 * kp, 2 * kp + 2)
                        nc.tensor.matmul(
                            ps[:, f, :], wo_sb[:, ksl, mb * P : (mb + 1) * P],
                            attn128[:, ksl, fsl],
                            start=(kp == 0), stop=(kp == KP - 1), perf_mode=DR)
                nc.vector.scalar_tensor_tensor(
                    out=zT[:, mb, :], in0=ps.rearrange("p f n -> p (f n)"),
                    scalar=1.0 / (WS * WS), in1=xpbo[:, mb, :],
                    op0=MUL, op1=ADD)
        wo_pool.close()

        # ================= LN2 (2 half-chunks of 256) + FFN =================
        with contextlib.ExitStack() as lnc:
            stat_ps = lnc.enter_context(tc.tile_pool(name="l2_stat", bufs=2, space="PSUM"))
            ap_ps = lnc.enter_context(tc.tile_pool(name="l2_ap", bufs=1, space="PSUM"))
            lns = lnc.enter_context(tc.tile_pool(name="l2_sq", bufs=3))
            lnr = lnc.enter_context(tc.tile_pool(name="l2_rows", bufs=2))
            tmp_p = lnc.enter_context(tc.tile_pool(name="l2_tmp", bufs=2))
            fps = lnc.enter_context(tc.tile_pool(name="ffn_ps", bufs=2, space="PSUM"))

            m_tiles, s_tiles, r_tiles = {}, {}, {}

            def l2_means(t0):
                sl = slice(t0 * 256, (t0 + 1) * 256)
                m_ps = stat_ps.tile([1, 256], F32, name="m_ps")
                for cb in range(CB):
                    nc.tensor.matmul(m_ps, ones_fr, zT[:, cb, sl],
                                     start=(cb == 0), stop=(cb == CB - 1))
                m_tiles[t0] = m_ps

            def l2_sqs(t0):
                sl = slice(t0 * 256, (t0 + 1) * 256)
                s_ps = stat_ps.tile([1, 256], F32, name="s_ps")
                for cb in range(CB):
                    sq = lns.tile([P, 256], F32R, name="sq")
                    nc.scalar.activation(sq, zT[:, cb, sl],
                                         mybir.ActivationFunctionType.Square)
                    nc.tensor.matmul(s_ps, ones_fr, sq,
                                     start=(cb == 0), stop=(cb == CB - 1))
                s_tiles[t0] = s_ps

            def l2_stats(t0):
                m_ps, s_ps = m_tiles.pop(t0), s_tiles.pop(t0)
                m_sb = lnr.tile([1, 256], F32, name="m_sb")
                nc.scalar.mul(m_sb, m_ps, 1.0 / C)
                var = lnr.tile([1, 256], F32, name="var")
                nc.scalar.mul(var, s_ps, 1.0 / C)
                msq = lnr.tile([1, 256], F32, name="msq")
                nc.vector.tensor_mul(msq, m_sb, m_sb)
                nc.vector.tensor_sub(var, var, msq)
                nc.scalar.activation(var, var, mybir.ActivationFunctionType.Sqrt,
                                     bias=eps_t)
                rstd = lnr.tile([1, 256], BF16, name="rstd")
                with nc.allow_low_precision(reason="f32r rstd"):
                    nc.vector.reciprocal(rstd, var)
                nm = lnr.tile([1, 256], BF16, name="nm")
                nc.vector.tensor_mul(nm, m_sb, rstd)
                nc.scalar.mul(nm, nm, -1.0)
                r_tiles[t0] = (rstd, nm)

            def l2_apply(t0):
                sl = slice(t0 * 256, (t0 + 1) * 256)
                rstd, nm = r_tiles.pop(t0)
                g_row = g_rows[:, 1, :]
                sc_ps = ap_ps.tile([P, 256], F32, name="sc_ps")
                bi_ps = ap_ps.tile([P, 256], F32, name="bi_ps")
                for cb in range(CB):
                    csl = slice(cb * P, (cb + 1) * P)
                    nc.tensor.matmul(sc_ps, g_row[:, csl], rstd, start=True, stop=True)
                    nc.tensor.matmul(bi_ps, g_row[:, csl], nm, start=True, stop=True)
                    tmp = tmp_p.tile([P, 256], F32R, name=f"tmp{cb % 2}")
                    nc.vector.tensor_mul(tmp, zT[:, cb, sl], sc_ps)
                    nc.vector.scalar_tensor_tensor(
                        out=h2T[:, cb, sl], in0=tmp,
                        scalar=b2_pc[:, cb : cb + 1], in1=bi_ps,
                        op0=ADD, op1=ADD)

            def ffn1_fb(fb, wtile, fi):
                ps = fps.tile([P, 2, 256], F32, name="ps_a")
                for t0 in range(2):
                    sl = slice(t0 * 256, (t0 + 1) * 256)
                    for kb in range(CB):
                        nc.tensor.matmul(ps[:, t0, :],
                                         wtile[:, kb, fi * P : (fi + 1) * P],
                                         h2T[:, kb, sl],
                                         start=(kb == 0), stop=(kb == CB - 1))
                nc.scalar.activation(aT[:, fb, :],
                                     ps.rearrange("p f n -> p (f n)"),
                                     mybir.ActivationFunctionType.Relu,
                                     bias=bf1_pc[:, fb : fb + 1])

            l2_means(0)
            l2_sqs(0)
            l2_means(1)
            l2_stats(0)
            l2_sqs(1)
            l2_apply(0)
            l2_stats(1)
            l2_apply(1)

            for wg in range(FB // 4):
                wtile = w1c.tile([P, CB, 4 * P], BF16, name="w1_c", bufs=2)
                nc.sync.dma_start(
                    out=wtile,
                    in_=w1[:, :].rearrange("(k p) n -> p k n", p=P)[
                        :, :, wg * 4 * P : (wg + 1) * 4 * P])
                for fi in range(4):
                    ffn1_fb(wg * 4 + fi, wtile, fi)
        w1pool.close()

        with contextlib.ExitStack() as p6:
            fps2 = p6.enter_context(tc.tile_pool(name="ffn2_ps", bufs=3, space="PSUM"))
            w2c = p6.enter_context(tc.tile_pool(name="w2c", bufs=2))
            outp = p6.enter_context(tc.tile_pool(name="outp", bufs=2))
            for mg in range(CB // 2):
                w2_c = w2c.tile([P, FB, 2 * P], BF16, name="w2_c", bufs=2)
                nc.sync.dma_start(
                    out=w2_c,
                    in_=w2[:, :].rearrange("(k p) n -> p k n", p=P)[
                        :, :, mg * 2 * P : (mg + 1) * 2 * P])
                for mi in range(2):
                    mb = mg * 2 + mi
                    ps = fps2.tile([P, TQ], F32, name="ps_o")
                    for kb in range(FB):
                        nc.tensor.matmul(ps, w2_c[:, kb, mi * P : (mi + 1) * P],
                                         aT[:, kb, :],
                                         start=(kb == 0), stop=(kb == FB - 1))
                    o_sb = outp.tile([P, TQ], F32, name="o_sb")
                    nc.vector.scalar_tensor_tensor(
                        out=o_sb, in0=ps, scalar=bf2_pc[:, mb : mb + 1],
                        in1=zT[:, mb, :], op0=ADD, op1=ADD)
                    nc.sync.dma_start(
                        out=outT[:, :].rearrange("(k p) t -> p k t", p=P)[:, mb, :],
                        in_=o_sb)

        # release singleton tiles in LIFO order
        _f_aT()
        _f_h2()
        _f_zT()
        _f_a128()
        _att[1][1]()
        _att[0][1]()
        _f_v()
        _f_kT()
        _f_qT()
        _f_xpbo()
    nc.compile()
    return nc


_CACHE = {}


def _get_built():
    if "nc" not in _CACHE:
        _CACHE["nc"] = build_kernel()
    return _CACHE["nc"]


def _qidx(j):
    return np.concatenate([np.arange((4 * i + j) * P, (4 * i + j + 1) * P)
                           for i in range(4)])


def _perm_times(j):
    own = [4 * i + j for i in range(4)]
    other = sorted(set(range(NSB)) - set(own))
    return own + other


def _build_in_maps(x, wq, wk, wv, wo, bo, g1, b1, g2, b2, w1, bf1, w2, bf2):
    f = np.float32
    bf = ml_dtypes.bfloat16
    f8 = ml_dtypes.float8_e4m3
    x = np.asarray(x, f)
    wq_m = (np.asarray(wq, f).transpose(1, 0, 2).reshape(C, C) * WS).astype(f8)
    wk_m = (np.asarray(wk, f).transpose(1, 0, 2).reshape(C, C) * WS).astype(f8)
    wv_m = (np.asarray(wv, f).transpose(1, 0, 2).reshape(C, C) * WS).astype(f8)
    wo_m = (np.asarray(wo, f) * WS).astype(f8)
    w1_m = np.asarray(w1, f).astype(bf)
    w2_m = np.asarray(w2, f).astype(bf)
    gb_m = np.ascontiguousarray(np.stack([np.asarray(a, f) for a in
                                          (g1, b1, g2, b2, bo, bf2)]))
    bf1_m = np.ascontiguousarray(np.asarray(bf1, f))

    in_maps = []
    for c in range(8):
        b, j = divmod(c, 4)
        ptimes = _perm_times(j)
        tok = np.concatenate([np.arange(t * P, (t + 1) * P) for t in ptimes])
        xT = np.ascontiguousarray(x[b].T[:, tok]).astype(bf)
        # boundary mask per key block kb: q block QL[kb]//P vs key time
        pp = np.arange(P)[:, None]
        cc = np.arange(P)[None, :]
        maskT = np.empty((P, NSB, P), f)
        for kb in range(NSB):
            qt = 4 * (QL[kb] // P) + j
            kt = ptimes[kb]
            maskT[:, kb, :] = ((qt - kt) * P + cc >= pp).astype(f)
        in_maps.append({
            "xT": xT, "maskT": maskT.astype(f8),
            "wq8": wq_m, "wk8": wk_m, "wv8": wv_m, "wo8": wo_m,
            "w1": w1_m, "w2": w2_m, "gb": gb_m, "bf1": bf1_m,
            "ones_in": np.ones((1, P), np.float32),
        })
    return in_maps


def _gather(results):
    out = np.empty((B, T, C), np.float32)
    for c in range(8):
        b, j = divmod(c, 4)
        out[b, _qidx(j)] = results[c]["outT"].T
    return out


def kernel(**inputs):
    in_maps = _build_in_maps(**inputs)
    nc = _get_built()
    res = run_bass_kernel_spmd(nc, in_maps, core_ids=list(range(8)))
    return _gather(res.results)


def run_traced(**inputs):
    in_maps = _build_in_maps(**inputs)
    nc = _get_built()
    return run_bass_kernel_spmd(nc, in_maps, core_ids=list(range(8)), trace=True)


# revision 20
# speedup vs baseline: 1.9302x; 1.1504x over previous
"""Trainium2 Bass kernel for a dense transformer decoder block (v2).

Sharding: pure data-parallel over 8 cores. Core c=(b*4+j) owns batch b and
query blocks {4i+j : i=0..3}. Host PERMUTES each core's 2048 tokens so the
core's own 512 query tokens come first; causality is enforced by per-core
per-key-block boundary masks, so the device program is identical on all
cores (j only affects host-prepared data).

Numerics (rms rel err budget ~2.3e-3 vs 2e-2 gate, verified by host emu):
- x, K/Q in bf16; probs/V/attn in fp8e4 (attention output is ~1.5% of the
  residual stream here: scores carry C**-0.5 scaling so softmax is nearly
  uniform -> fp8 noise in the attention path is negligible downstream).
- Q/K/V and wo projections run fp8e4 with DoubleRow perf mode (2 x 128-deep
  contraction per matmul at 0.5 cycles/row). Weights host-scaled x32; the
  scale folds into the softmax scale (1/1024), cancels in the softmax
  normalize, and is divided out in the z epilogue (1/1024).
- FFN stays bf16 (fp8 there costs ~1e-2 rms - too close to the gate).

Cost-model facts this build exploits (instruction_cost_v2.rs):
- matmul time = out_free_size * pe_cycle * cpr; cpr: bf16/f32r(>=256 free)=1.0,
  fp8 DoubleRow=0.5 (with 256-deep contraction -> 4x bf16 FLOP rate).
- PE p-state ramps with *continuous* busy time (low 1.54ns/row after idle,
  0.83 mid, 0.42 full after 3us) -> LN statistic matmuls are emitted batched
  and interleaved so the PE never sleeps between them.
- Collectives cost 15us fixed + 40GB/s -> no collectives; duplicated
  K/V projection is cheap in fp8-DoubleRow instead.
"""

import contextlib

import numpy as np
import ml_dtypes

import concourse.bass as bass
import concourse.bacc as bacc
import concourse.mybir as mybir
import concourse.tile as tile
from concourse.bass_utils import run_bass_kernel_spmd

B, T, C, H, HD, F = 2, 2048, 1024, 16, 64, 4096
EPS = 1e-5
P = 128
CB = C // P          # 8 emb chunks
KP = CB // 2         # 4 DoubleRow chunk-pairs
FB = F // P          # 32 ffn chunks
TQ = 512             # own query tokens per core
TKV = 2048           # kv tokens (full batch, permuted: own 512 first)
NSB = TKV // P       # 16 key blocks
WS = 32.0            # host weight scale for fp8 (wq/wk/wv/wo/w1)
WS2 = 64.0           # host weight scale for fp8 w2
SCALE2 = float(C) ** -0.5 / (WS * WS)   # folds both x32 into softmax scale
LOG32 = float(np.log(32.0))             # probs8 = 32*exp(score): fp8 range

F32 = mybir.dt.float32
F32R = mybir.dt.float32r
BF16 = mybir.dt.bfloat16
FP8 = mybir.dt.float8e4
DR = mybir.MatmulPerfMode.DoubleRow
ADD = mybir.AluOpType.add
MUL = mybir.AluOpType.mult

# q_lo per key block kb (j-independent under the own-first permutation):
# own blocks kb<4 sit at q block kb; non-own block n=kb-4 needs q blocks
# i >= n//3 (boundary block handled by mask).
QL = [kb * P if kb < 4 else ((kb - 4) // 3) * P for kb in range(NSB)]


def build_kernel():
    nc = bacc.Bacc("TRN2", num_devices=8)

    xT = nc.dram_tensor("xT", [C, TKV], BF16, kind="ExternalInput")
    maskT = nc.dram_tensor("maskT", [P, NSB, P], FP8, kind="ExternalInput")
    wq8 = nc.dram_tensor("wq8", [C, C], FP8, kind="ExternalInput")
    wk8 = nc.dram_tensor("wk8", [C, C], FP8, kind="ExternalInput")
    wv8 = nc.dram_tensor("wv8", [C, C], FP8, kind="ExternalInput")
    wo8 = nc.dram_tensor("wo8", [C, C], FP8, kind="ExternalInput")
    w1 = nc.dram_tensor("w1", [C, F], FP8, kind="ExternalInput")
    w2 = nc.dram_tensor("w2", [F, C], FP8, kind="ExternalInput")
    gb = nc.dram_tensor("gb", [6, C], F32R, kind="ExternalInput")
    bf1 = nc.dram_tensor("bf1", [F], F32, kind="ExternalInput")
    ones_in = nc.dram_tensor("ones_in", [1, P], F32R, kind="ExternalInput")
    outT = nc.dram_tensor("outT", [C, TQ], F32, kind="ExternalOutput")

    with tile.TileContext(nc) as tc, contextlib.ExitStack() as ctx:
        singles = ctx.enter_context(tc.tile_pool(name="singles", bufs=1))

        ones_fr = singles.tile([P, 1], F32R)
        nc.sync.dma_start(out=ones_fr, in_=ones_in[:, 0:1].to_broadcast([P, 1]))
        ones_bf = singles.tile([P, 1], BF16)
        nc.vector.memset(ones_bf, 1.0)
        eps_t = singles.tile([1, 1], F32)
        nc.vector.memset(eps_t, EPS)
        log32_t = singles.tile([P, 1], F32)
        nc.vector.memset(log32_t, LOG32)

        g_rows = singles.tile([1, 2, C], BF16)
        nc.gpsimd.dma_start(out=g_rows[:, 0, :], in_=gb[None, 0, :].bitcast(F32))
        nc.gpsimd.dma_start(out=g_rows[:, 1, :], in_=gb[None, 2, :].bitcast(F32))
        b1_pc = singles.tile([P, CB], F32)
        nc.sync.dma_start(out=b1_pc, in_=gb[1, :].rearrange("(k p) -> p k", p=P).bitcast(F32))
        b2_pc = singles.tile([P, CB], F32)
        nc.sync.dma_start(out=b2_pc, in_=gb[3, :].rearrange("(k p) -> p k", p=P).bitcast(F32))
        bo_pc = singles.tile([P, CB], F32)
        nc.sync.dma_start(out=bo_pc, in_=gb[4, :].rearrange("(k p) -> p k", p=P).bitcast(F32))
        bf2_pc = singles.tile([P, CB], F32)
        nc.sync.dma_start(out=bf2_pc, in_=gb[5, :].rearrange("(k p) -> p k", p=P).bitcast(F32))
        bf1_pc = singles.tile([P, FB], F32)
        nc.sync.dma_start(out=bf1_pc, in_=bf1[:].rearrange("(k p) -> p k", p=P))
        mask_sb = singles.tile([P, NSB, P], FP8)
        nc.sync.dma_start(out=mask_sb, in_=maskT[:, :, :])

        # --- persistent activation tiles (alloc order = reverse free order) ---
        xpbo, _f_xpbo = tc.tile([P, CB, TQ], BF16, name="xpbo")  # x + bo (own)
        qT, _f_qT = tc.tile([P, CB, TQ], FP8, name="qT")         # 32*q
        kT, _f_kT = tc.tile([P, CB, TKV], FP8, name="kT")        # 32*k
        v_aug, _f_v = tc.tile([P, NSB, H, HD + 1], FP8, name="v_aug")
        nc.vector.memset(v_aug[:, :, :, HD], 1.0)
        _att = [tc.tile([HD, H // 2, TQ], FP8, name=f"attn8u{u}")
                for u in range(2)]
        attn8u = [t for t, _ in _att]
        attn128, _f_a128 = tc.tile([P, H // 2, TQ], FP8, name="attn128")
        h8, free_h8 = tc.tile([P, CB, TKV], FP8, name="h8")
        w8pool = contextlib.ExitStack()
        w8p = w8pool.enter_context(tc.tile_pool(name="w8p", bufs=1))
        wq_sb = w8p.tile([P, CB, C], FP8, name="wq_sb")
        wk_sb = w8p.tile([P, CB, C], FP8, name="wk_sb")
        wv_sb = w8p.tile([P, CB, C], FP8, name="wv_sb")
        x_sb, free_x = tc.tile([P, CB, TKV], BF16, name="x_sb")

        for cb in range(CB):
            nc.sync.dma_start(
                out=x_sb[:, cb, :],
                in_=xT[:, :].rearrange("(k p) t -> p k t", p=P)[:, cb, :])
        nc.sync.dma_start(out=wq_sb, in_=wq8[:, :].rearrange("(k p) n -> p k n", p=P))
        nc.sync.dma_start(out=wk_sb, in_=wk8[:, :].rearrange("(k p) n -> p k n", p=P))
        nc.sync.dma_start(out=wv_sb, in_=wv8[:, :].rearrange("(k p) n -> p k n", p=P))

        # xpbo = x + bo on own columns (feeds the z residual later)
        for cb in range(CB):
            eng = nc.vector if cb % 2 == 0 else nc.gpsimd
            eng.tensor_scalar_add(xpbo[:, cb, :], x_sb[:, cb, 0:TQ],
                                  bo_pc[:, cb : cb + 1])

        # ================= LN1 over all TKV tokens -> h8 (fp8) =================
        NCH = TKV // TQ  # 4 chunks of 512
        with contextlib.ExitStack() as lnc:
            stat_ps = lnc.enter_context(tc.tile_pool(name="ln_stat", bufs=2, space="PSUM"))
            ap_ps = lnc.enter_context(tc.tile_pool(name="ln_ap", bufs=1, space="PSUM"))
            lns = lnc.enter_context(tc.tile_pool(name="ln_sq", bufs=3))
            lnr = lnc.enter_context(tc.tile_pool(name="ln_rows", bufs=2))
            tmp_p = lnc.enter_context(tc.tile_pool(name="ln_tmp", bufs=2))

            m_tiles, s_tiles, r_tiles = {}, {}, {}

            def ln_means(t0, src, ntok):
                sl = slice(t0 * ntok, (t0 + 1) * ntok)
                m_ps = stat_ps.tile([1, ntok], F32, name="m_ps")
                for cb in range(CB):
                    nc.tensor.matmul(m_ps, ones_bf, src[:, cb, sl],
                                     start=(cb == 0), stop=(cb == CB - 1))
                m_tiles[t0] = m_ps

            def ln_sqs(t0, src, ntok, sq_dt=BF16, ones=None):
                sl = slice(t0 * ntok, (t0 + 1) * ntok)
                s_ps = stat_ps.tile([1, ntok], F32, name="s_ps")
                for cb in range(CB):
                    sq = lns.tile([P, ntok], F32R, name="sq")
                    nc.scalar.activation(sq, src[:, cb, sl],
                                         mybir.ActivationFunctionType.Square)
                    nc.tensor.matmul(s_ps, ones_fr, sq,
                                     start=(cb == 0), stop=(cb == CB - 1))
                s_tiles[t0] = s_ps

            def ln_stats(t0, ntok):
                m_ps, s_ps = m_tiles.pop(t0), s_tiles.pop(t0)
                m_sb = lnr.tile([1, ntok], F32, name="m_sb")
                nc.scalar.mul(m_sb, m_ps, 1.0 / C)
                var = lnr.tile([1, ntok], F32, name="var")
                nc.scalar.mul(var, s_ps, 1.0 / C)
                msq = lnr.tile([1, ntok], F32, name="msq")
                nc.vector.tensor_mul(msq, m_sb, m_sb)
                nc.vector.tensor_sub(var, var, msq)
                nc.scalar.activation(var, var, mybir.ActivationFunctionType.Sqrt,
                                     bias=eps_t)
                rstd = lnr.tile([1, ntok], BF16, name="rstd")
                with nc.allow_low_precision(reason="f32r rstd"):
                    nc.vector.reciprocal(rstd, var)
                nm = lnr.tile([1, ntok], BF16, name="nm")
                nc.vector.tensor_mul(nm, m_sb, rstd)
                nc.scalar.mul(nm, nm, -1.0)
                r_tiles[t0] = (rstd, nm)

            def ln_apply(t0, src, dst, ntok, g_row, b_pc):
                sl = slice(t0 * ntok, (t0 + 1) * ntok)
                rstd, nm = r_tiles.pop(t0)
                sc_ps = ap_ps.tile([P, ntok], F32, name="sc_ps")
                bi_ps = ap_ps.tile([P, ntok], F32, name="bi_ps")
                for cb in range(CB):
                    csl = slice(cb * P, (cb + 1) * P)
                    nc.tensor.matmul(sc_ps, g_row[:, csl], rstd, start=True, stop=True)
                    nc.tensor.matmul(bi_ps, g_row[:, csl], nm, start=True, stop=True)
                    tmp = tmp_p.tile([P, ntok], F32R, name=f"tmp{cb % 2}")
                    nc.vector.tensor_mul(tmp, src[:, cb, sl], sc_ps)
                    nc.vector.scalar_tensor_tensor(
                        out=dst[:, cb, sl], in0=tmp,
                        scalar=b_pc[:, cb : cb + 1], in1=bi_ps,
                        op0=ADD, op1=ADD)

            g1_row = g_rows[:, 0, :]
            # interleave so the PE pipeline never drains
            ln_means(0, x_sb, TQ)
            ln_sqs(0, x_sb, TQ)
            ln_means(1, x_sb, TQ)
            ln_stats(0, TQ)
            ln_sqs(1, x_sb, TQ)
            ln_apply(0, x_sb, h8, TQ, g1_row, b1_pc)
            ln_means(2, x_sb, TQ)
            ln_stats(1, TQ)
            ln_sqs(2, x_sb, TQ)
            ln_apply(1, x_sb, h8, TQ, g1_row, b1_pc)
            ln_means(3, x_sb, TQ)
            ln_stats(2, TQ)
            ln_sqs(3, x_sb, TQ)
            ln_apply(2, x_sb, h8, TQ, g1_row, b1_pc)
            ln_stats(3, TQ)
            ln_apply(3, x_sb, h8, TQ, g1_row, b1_pc)

            # ---- Q projection (own 512 tokens), fp8 DoubleRow ----
            with contextlib.ExitStack() as pq:
                qps = pq.enter_context(tc.tile_pool(name="q_ps", bufs=2, space="PSUM"))
                for mb in range(CB):
                    ps = qps.tile([P, 2, 256], F32, name="ps_q")
                    for f in range(2):
                        fsl = slice(f * 256, (f + 1) * 256)
                        for kp in range(KP):
                            ksl = slice(2 * kp, 2 * kp + 2)
                            nc.tensor.matmul(
                                ps[:, f, :], wq_sb[:, ksl, mb * P : (mb + 1) * P],
                                h8[:, ksl, fsl],
                                start=(kp == 0), stop=(kp == KP - 1), perf_mode=DR)
                    nc.scalar.mul(qT[:, mb, :], ps.rearrange("p f n -> p (f n)"), 1.0)

        # ---- K/V projections (full 2048 tokens), fp8 DoubleRow ----
        with contextlib.ExitStack() as pkv:
            kps = pkv.enter_context(tc.tile_pool(name="k_ps", bufs=6, space="PSUM"))
            vps = pkv.enter_context(tc.tile_pool(name="v_ps", bufs=2, space="PSUM"))
            for mb in range(CB):
                for tf in range(TKV // 256):
                    fsl = slice(tf * 256, (tf + 1) * 256)
                    ps = kps.tile([P, 256], F32, name="ps_k")
                    for kp in range(KP):
                        ksl = slice(2 * kp, 2 * kp + 2)
                        nc.tensor.matmul(
                            ps, wk_sb[:, ksl, mb * P : (mb + 1) * P],
                            h8[:, ksl, fsl],
                            start=(kp == 0), stop=(kp == KP - 1), perf_mode=DR)
                    eng = nc.scalar if tf % 2 == 0 else None
                    if eng is nc.scalar:
                        nc.scalar.mul(kT[:, mb, fsl], ps, 1.0)
                    else:
                        nc.vector.tensor_copy(kT[:, mb, fsl], ps)
            for tb in range(NSB):
                for hh in range(2):  # 8-head halves of the hd axis
                    ps = vps.tile([P, 2, 256], F32, name="ps_v")
                    for f in range(2):
                        fsl = slice(hh * 512 + f * 256, hh * 512 + (f + 1) * 256)
                        for kp in range(KP):
                            ksl = slice(2 * kp, 2 * kp + 2)
                            nc.tensor.matmul(
                                ps[:, f, :],
                                h8[:, ksl, tb * P : (tb + 1) * P],
                                wv_sb[:, ksl, fsl],
                                start=(kp == 0), stop=(kp == KP - 1), perf_mode=DR)
                    if tb % 2 == 0:
                        nc.vector.tensor_copy(
                            v_aug[:, tb, hh * 8 : (hh + 1) * 8, 0:HD],
                            ps.rearrange("p f (h d) -> p (f h) d", d=HD))
                    else:
                        nc.scalar.mul(
                            v_aug[:, tb, hh * 8 : (hh + 1) * 8, 0:HD],
                            ps.rearrange("p f (h d) -> p (f h) d", d=HD), 1.0)
        free_x()
        w8pool.close()
        free_h8()

        zT, _f_zT = tc.tile([P, CB, TQ], F32R, name="zT")
        h2T, _f_h2 = tc.tile([P, CB, TQ], BF16, name="h2T")
        h2q8, _f_h2q = tc.tile([P, CB, TQ], FP8, name="h2q8")
        h2r8, _f_h2r = tc.tile([P, CB, TQ], FP8, name="h2r8")
        aT8, _f_a8 = tc.tile([P, FB, TQ], FP8, name="aT8")
        aTr8, _f_ar8 = tc.tile([P, FB, TQ], FP8, name="aTr8")

        # prefetch wo and first FFN1 weights during attention
        # (w1c opens first: pools are a LIFO stack and wo_p closes earlier)
        w1pool = contextlib.ExitStack()
        w1c = w1pool.enter_context(tc.tile_pool(name="w1c", bufs=2))
        wo_pool = contextlib.ExitStack()
        wo_sb = wo_pool.enter_context(tc.tile_pool(name="wo_p", bufs=1)).tile(
            [P, CB, C], FP8, name="wo_sb")
        nc.sync.dma_start(out=wo_sb, in_=wo8[:, :].rearrange("(k p) n -> p k n", p=P))

        # ================= attention (per head pair) =================
        with contextlib.ExitStack() as p3:
            sc_ps_pool = p3.enter_context(tc.tile_pool(name="sc_ps", bufs=2, space="PSUM"))
            pair_ps_pool = p3.enter_context(tc.tile_pool(name="pair_ps", bufs=2, space="PSUM"))
            bc_pool = p3.enter_context(tc.tile_pool(name="bc", bufs=2))
            probs_pool = p3.enter_context(tc.tile_pool(name="probs", bufs=2))
            rec_pool = p3.enter_context(tc.tile_pool(name="rec", bufs=2))

            # key-block pairs (2-bank batched exp; boundary overcompute is
            # masked/unread)
            SBG = [[0, 1], [2, 3], [4, 5], [6, 7], [8, 9], [10, 11],
                   [12, 13], [14, 15]]
            for pair in range(H // 2):
                probs8 = probs_pool.tile([P, 2, NSB, TQ], FP8, name="probs8")
                ps_h = [pair_ps_pool.tile([HD + 1, TQ], F32, name=f"ps_h{u}")
                        for u in range(2)]
                for grp in SBG:
                    q0 = QL[grp[0]]
                    n = len(grp)
                    for u in range(2):
                        prow = slice(u * HD, (u + 1) * HD)
                        ps3 = sc_ps_pool.tile([P, 2, TQ], F32, name="ps_s")
                        for i, sb in enumerate(grp):
                            qi = QL[sb]
                            nc.tensor.matmul(
                                ps3[:, i, qi:TQ],
                                kT[prow, pair, sb * P : (sb + 1) * P],
                                qT[prow, pair, qi:TQ],
                                start=True, stop=True)
                        nc.scalar.activation(
                            probs8[:, u, grp[0] : grp[0] + n, q0:TQ],
                            ps3[:, 0:n, q0:TQ],
                            mybir.ActivationFunctionType.Exp,
                            scale=SCALE2, bias=log32_t)
                        for sb in grp:
                            qb = QL[sb]
                            eng = nc.gpsimd if sb % 3 == 2 else nc.vector
                            eng.tensor_mul(
                                probs8[:, u, sb, qb : qb + P],
                                probs8[:, u, sb, qb : qb + P],
                                mask_sb[:, sb, :])
                for sb in range(NSB):
                    first, last = (sb == 0), (sb == NSB - 1)
                    for u in range(2):
                        nc.tensor.matmul(
                            ps_h[u][:, QL[sb]:TQ],
                            v_aug[:, sb, 2 * pair + u, :],
                            probs8[:, u, sb, QL[sb]:TQ],
                            start=first, stop=last)
                rec_pair = rec_pool.tile([1, 2, TQ], BF16, name="rec_pair")
                for u in range(2):
                    with nc.allow_low_precision(reason="softmax denom"):
                        nc.vector.reciprocal(rec_pair[:, u, :],
                                             ps_h[u][HD : HD + 1, :])
                bc_sb = bc_pool.tile([HD, 2, TQ], BF16, name="bc_sb")
                nc.gpsimd.partition_broadcast(bc_sb, rec_pair)
                for u in range(2):
                    nc.vector.tensor_mul(attn8u[u][:, pair, :],
                                         ps_h[u][0:HD, :], bc_sb[:, u, :])
                # stack this pair onto 128 partitions early (SBUF->SBUF DMA)
                nc.sync.dma_start(out=attn128[0:HD, pair, :],
                                  in_=attn8u[0][:, pair, :])
                nc.sync.dma_start(out=attn128[HD:P, pair, :],
                                  in_=attn8u[1][:, pair, :])

        # ========== wo (fp8 DR) + residual -> zT, LN2 fused into the loop ==========
        with contextlib.ExitStack() as p4:
            ops = p4.enter_context(tc.tile_pool(name="wo_ps", bufs=2, space="PSUM"))
            stat_ps = p4.enter_context(tc.tile_pool(name="l2_stat", bufs=1, space="PSUM"))
            ap_ps = p4.enter_context(tc.tile_pool(name="l2_ap", bufs=1, space="PSUM"))
            lns = p4.enter_context(tc.tile_pool(name="l2_sq", bufs=3))
            lnr = p4.enter_context(tc.tile_pool(name="l2_rows", bufs=1))
            tmp_p = p4.enter_context(tc.tile_pool(name="l2_tmp", bufs=2))
            ab_p = p4.enter_context(tc.tile_pool(name="ab", bufs=2))
            fps = p4.enter_context(tc.tile_pool(name="ffn_ps", bufs=2, space="PSUM"))

            m_ps = stat_ps.tile([1, TQ], F32, name="m_ps")
            s_ps = stat_ps.tile([1, TQ], F32, name="s_ps")

            def wo_mb(mb):
                ps = ops.tile([P, 2, 256], F32, name="ps_z")
                for f in range(2):
                    fsl = slice(f * 256, (f + 1) * 256)
                    for kp in range(KP):
                        ksl = slice(2 * kp, 2 * kp + 2)
                        nc.tensor.matmul(
                            ps[:, f, :], wo_sb[:, ksl, mb * P : (mb + 1) * P],
                            attn128[:, ksl, fsl],
                            start=(kp == 0), stop=(kp == KP - 1), perf_mode=DR)
                nc.vector.scalar_tensor_tensor(
                    out=zT[:, mb, :], in0=ps.rearrange("p f n -> p (f n)"),
                    scalar=1.0 / (WS * WS), in1=xpbo[:, mb, :],
                    op0=MUL, op1=ADD)

            def l2_mean(cb):
                nc.tensor.matmul(m_ps, ones_fr, zT[:, cb, :],
                                 start=(cb == 0), stop=(cb == CB - 1),
                                 skip_group_check=True)

            def l2_sq(cb):
                sq = lns.tile([P, TQ], F32R, name="sq")
                nc.scalar.activation(sq, zT[:, cb, :],
                                     mybir.ActivationFunctionType.Square)
                nc.tensor.matmul(s_ps, ones_fr, sq,
                                 start=(cb == 0), stop=(cb == CB - 1),
                                 skip_group_check=True)

            for mb in range(CB):
                wo_mb(mb)
                if mb >= 1:
                    l2_mean(mb - 1)
                if mb >= 2:
                    l2_sq(mb - 2)
            l2_mean(CB - 1)
            l2_sq(CB - 2)
            l2_sq(CB - 1)

            m_sb = lnr.tile([1, TQ], F32, name="m_sb")
            nc.scalar.mul(m_sb, m_ps, 1.0 / C)
            var = lnr.tile([1, TQ], F32, name="var")
            nc.scalar.mul(var, s_ps, 1.0 / C)
            msq = lnr.tile([1, TQ], F32, name="msq")
            nc.vector.tensor_mul(msq, m_sb, m_sb)
            nc.vector.tensor_sub(var, var, msq)
            nc.scalar.activation(var, var, mybir.ActivationFunctionType.Sqrt,
                                 bias=eps_t)
            rstd = lnr.tile([1, TQ], BF16, name="rstd")
            with nc.allow_low_precision(reason="f32r rstd"):
                nc.vector.reciprocal(rstd, var)
            nm = lnr.tile([1, TQ], BF16, name="nm")
            nc.vector.tensor_mul(nm, m_sb, rstd)
            nc.scalar.mul(nm, nm, -1.0)

            g_row = g_rows[:, 1, :]
            sc_ps = ap_ps.tile([P, TQ], F32, name="sc_ps")
            bi_ps = ap_ps.tile([P, TQ], F32, name="bi_ps")
            for cb in range(CB):
                csl = slice(cb * P, (cb + 1) * P)
                nc.tensor.matmul(sc_ps, g_row[:, csl], rstd, start=True, stop=True)
                nc.tensor.matmul(bi_ps, g_row[:, csl], nm, start=True, stop=True)
                tmp = tmp_p.tile([P, TQ], F32R, name="tmp")
                nc.vector.tensor_mul(tmp, zT[:, cb, :], sc_ps)
                nc.vector.scalar_tensor_tensor(
                    out=h2T[:, cb, :], in0=tmp,
                    scalar=b2_pc[:, cb : cb + 1], in1=bi_ps,
                    op0=ADD, op1=ADD)
                nc.scalar.mul(h2q8[:, cb, :], h2T[:, cb, :], 1.0)
                nc.vector.tensor_sub(h2r8[:, cb, :], h2T[:, cb, :],
                                     h2q8[:, cb, :])
                # fold bf2 into zT now that LN2 is done with it
                nc.gpsimd.tensor_scalar_add(zT[:, cb, :], zT[:, cb, :],
                                            bf2_pc[:, cb : cb + 1])

            # ---------------- FFN1: (h2q8 + h2r8) @ w1 (fp8 DR) ----------------
            for wg in range(FB // 4):
                wtile = w1c.tile([P, CB, 4 * P], FP8, name="w1_c", bufs=2)
                nc.sync.dma_start(
                    out=wtile,
                    in_=w1[:, :].rearrange("(k p) n -> p k n", p=P)[
                        :, :, wg * 4 * P : (wg + 1) * 4 * P])
                for fi in range(4):
                    fb = wg * 4 + fi
                    ps = fps.tile([P, 2, 256], F32, name="ps_a")
                    for t0 in range(2):
                        tsl = slice(t0 * 256, (t0 + 1) * 256)
                        for ti, src8 in enumerate((h2q8, h2r8)):
                            for kp in range(KP):
                                ksl = slice(2 * kp, 2 * kp + 2)
                                nc.tensor.matmul(
                                    ps[:, t0, :],
                                    wtile[:, ksl, fi * P : (fi + 1) * P],
                                    src8[:, ksl, tsl],
                                    start=(ti == 0 and kp == 0),
                                    stop=(ti == 1 and kp == KP - 1),
                                    perf_mode=DR)
                    aTb = ab_p.tile([P, TQ], BF16, name="aTb")
                    nc.scalar.activation(aTb, ps.rearrange("p f n -> p (f n)"),
                                         mybir.ActivationFunctionType.Relu,
                                         scale=1.0 / WS,
                                         bias=bf1_pc[:, fb : fb + 1])
                    nc.scalar.mul(aT8[:, fb, :], aTb, 1.0)
                    nc.vector.tensor_sub(aTr8[:, fb, :], aTb, aT8[:, fb, :])
        wo_pool.close()
        w1pool.close()

        # ---------------- FFN2: (aT8 + aTr8) @ w2 (fp8 DR) ----------------
        with contextlib.ExitStack() as p6:
            fps2 = p6.enter_context(tc.tile_pool(name="ffn2_ps", bufs=3, space="PSUM"))
            w2c = p6.enter_context(tc.tile_pool(name="w2c", bufs=2))
            outp = p6.enter_context(tc.tile_pool(name="outp", bufs=2))
            for mb in range(CB):
                w2_c = w2c.tile([P, FB, P], FP8, name="w2_c", bufs=2)
                nc.sync.dma_start(
                    out=w2_c,
                    in_=w2[:, :].rearrange("(k p) n -> p k n", p=P)[
                        :, :, mb * P : (mb + 1) * P])
                ps = fps2.tile([P, 2, 256], F32, name="ps_o")
                for t0 in range(2):
                    tsl = slice(t0 * 256, (t0 + 1) * 256)
                    for ti, src8 in enumerate((aT8, aTr8)):
                        for kp in range(FB // 2):
                            ksl = slice(2 * kp, 2 * kp + 2)
                            nc.tensor.matmul(
                                ps[:, t0, :], w2_c[:, ksl, :],
                                src8[:, ksl, tsl],
                                start=(ti == 0 and kp == 0),
                                stop=(ti == 1 and kp == FB // 2 - 1),
                                perf_mode=DR)
                o_sb = outp.tile([P, TQ], F32, name="o_sb")
                nc.vector.scalar_tensor_tensor(
                    out=o_sb, in0=ps.rearrange("p f n -> p (f n)"),
                    scalar=1.0 / WS2, in1=zT[:, mb, :],
                    op0=MUL, op1=ADD)
                nc.sync.dma_start(
                    out=outT[:, :].rearrange("(k p) t -> p k t", p=P)[:, mb, :],
                    in_=o_sb)

        # release singleton tiles in LIFO order
        _f_ar8()
        _f_a8()
        _f_h2r()
        _f_h2q()
        _f_h2()
        _f_zT()
        _f_a128()
        _att[1][1]()
        _att[0][1]()
        _f_v()
        _f_kT()
        _f_qT()
        _f_xpbo()
    nc.compile()
    return nc


_CACHE = {}


def _get_built():
    if "nc" not in _CACHE:
        _CACHE["nc"] = build_kernel()
    return _CACHE["nc"]


def _qidx(j):
    return np.concatenate([np.arange((4 * i + j) * P, (4 * i + j + 1) * P)
                           for i in range(4)])


def _perm_times(j):
    own = [4 * i + j for i in range(4)]
    other = sorted(set(range(NSB)) - set(own))
    return own + other


def _build_in_maps(x, wq, wk, wv, wo, bo, g1, b1, g2, b2, w1, bf1, w2, bf2):
    f = np.float32
    bf = ml_dtypes.bfloat16
    f8 = ml_dtypes.float8_e4m3
    x = np.asarray(x, f)
    wq_m = (np.asarray(wq, f).transpose(1, 0, 2).reshape(C, C) * WS).astype(f8)
    wk_m = (np.asarray(wk, f).transpose(1, 0, 2).reshape(C, C) * WS).astype(f8)
    wv_m = (np.asarray(wv, f).transpose(1, 0, 2).reshape(C, C) * WS).astype(f8)
    wo_m = (np.asarray(wo, f) * WS).astype(f8)
    w1_m = (np.asarray(w1, f) * WS).astype(f8)
    w2_m = (np.asarray(w2, f) * WS2).astype(f8)
    gb_m = np.ascontiguousarray(np.stack([np.asarray(a, f) for a in
                                          (g1, b1, g2, b2, bo, bf2)]))
    bf1_m = np.ascontiguousarray(np.asarray(bf1, f))

    in_maps = []
    for c in range(8):
        b, j = divmod(c, 4)
        ptimes = _perm_times(j)
        tok = np.concatenate([np.arange(t * P, (t + 1) * P) for t in ptimes])
        xT = np.ascontiguousarray(x[b].T[:, tok]).astype(bf)
        # boundary mask per key block kb: q block QL[kb]//P vs key time
        pp = np.arange(P)[:, None]
        cc = np.arange(P)[None, :]
        maskT = np.empty((P, NSB, P), f)
        for kb in range(NSB):
            qt = 4 * (QL[kb] // P) + j
            kt = ptimes[kb]
            maskT[:, kb, :] = ((qt - kt) * P + cc >= pp).astype(f)
        in_maps.append({
            "xT": xT, "maskT": maskT.astype(f8),
            "wq8": wq_m, "wk8": wk_m, "wv8": wv_m, "wo8": wo_m,
            "w1": w1_m, "w2": w2_m, "gb": gb_m, "bf1": bf1_m,
            "ones_in": np.ones((1, P), np.float32),
        })
    return in_maps


def _gather(results):
    out = np.empty((B, T, C), np.float32)
    for c in range(8):
        b, j = divmod(c, 4)
        out[b, _qidx(j)] = results[c]["outT"].T
    return out


def kernel(**inputs):
    in_maps = _build_in_maps(**inputs)
    nc = _get_built()
    res = run_bass_kernel_spmd(nc, in_maps, core_ids=list(range(8)))
    return _gather(res.results)


def run_traced(**inputs):
    in_maps = _build_in_maps(**inputs)
    nc = _get_built()
    return run_bass_kernel_spmd(nc, in_maps, core_ids=list(range(8)), trace=True)


# revision 24
# speedup vs baseline: 1.9943x; 1.0332x over previous
"""Trainium2 Bass kernel for a dense transformer decoder block (v2).

Sharding: pure data-parallel over 8 cores. Core c=(b*4+j) owns batch b and
query blocks {4i+j : i=0..3}. Host PERMUTES each core's 2048 tokens so the
core's own 512 query tokens come first; causality is enforced by per-core
per-key-block boundary masks, so the device program is identical on all
cores (j only affects host-prepared data).

Numerics (rms rel err budget ~2.3e-3 vs 2e-2 gate, verified by host emu):
- x, K/Q in bf16; probs/V/attn in fp8e4 (attention output is ~1.5% of the
  residual stream here: scores carry C**-0.5 scaling so softmax is nearly
  uniform -> fp8 noise in the attention path is negligible downstream).
- Q/K/V and wo projections run fp8e4 with DoubleRow perf mode (2 x 128-deep
  contraction per matmul at 0.5 cycles/row). Weights host-scaled x32; the
  scale folds into the softmax scale (1/1024), cancels in the softmax
  normalize, and is divided out in the z epilogue (1/1024).
- FFN stays bf16 (fp8 there costs ~1e-2 rms - too close to the gate).

Cost-model facts this build exploits (instruction_cost_v2.rs):
- matmul time = out_free_size * pe_cycle * cpr; cpr: bf16/f32r(>=256 free)=1.0,
  fp8 DoubleRow=0.5 (with 256-deep contraction -> 4x bf16 FLOP rate).
- PE p-state ramps with *continuous* busy time (low 1.54ns/row after idle,
  0.83 mid, 0.42 full after 3us) -> LN statistic matmuls are emitted batched
  and interleaved so the PE never sleeps between them.
- Collectives cost 15us fixed + 40GB/s -> no collectives; duplicated
  K/V projection is cheap in fp8-DoubleRow instead.
"""

import contextlib

import numpy as np
import ml_dtypes

import concourse.bass as bass
import concourse.bacc as bacc
import concourse.mybir as mybir
import concourse.tile as tile
from concourse.bass_utils import run_bass_kernel_spmd

B, T, C, H, HD, F = 2, 2048, 1024, 16, 64, 4096
EPS = 1e-5
P = 128
CB = C // P          # 8 emb chunks
KP = CB // 2         # 4 DoubleRow chunk-pairs
FB = F // P          # 32 ffn chunks
TQ = 512             # own query tokens per core
TKV = 2048           # kv tokens (full batch, permuted: own 512 first)
NSB = TKV // P       # 16 key blocks
WS = 32.0            # host weight scale for fp8 (wq/wk/wv/wo/w1)
WS2 = 64.0           # host weight scale for fp8 w2
SCALE2 = float(C) ** -0.5 / (WS * WS)   # folds both x32 into softmax scale
LOG32 = float(np.log(32.0))             # probs8 = 32*exp(score): fp8 range

F32 = mybir.dt.float32
F32R = mybir.dt.float32r
BF16 = mybir.dt.bfloat16
FP8 = mybir.dt.float8e4
DR = mybir.MatmulPerfMode.DoubleRow
ADD = mybir.AluOpType.add
MUL = mybir.AluOpType.mult

# q_lo per key block kb (j-independent under the own-first permutation):
# own blocks kb<4 sit at q block kb; non-own block n=kb-4 needs q blocks
# i >= n//3 (boundary block handled by mask).
QL = [kb * P if kb < 4 else ((kb - 4) // 3) * P for kb in range(NSB)]


def build_kernel():
    nc = bacc.Bacc("TRN2", num_devices=8)

    xT = nc.dram_tensor("xT", [C, TKV], BF16, kind="ExternalInput")
    maskT = nc.dram_tensor("maskT", [P, NSB, P], FP8, kind="ExternalInput")
    wq8 = nc.dram_tensor("wq8", [C, C], FP8, kind="ExternalInput")
    wk8 = nc.dram_tensor("wk8", [C, C], FP8, kind="ExternalInput")
    wv8 = nc.dram_tensor("wv8", [C, C], FP8, kind="ExternalInput")
    wo8 = nc.dram_tensor("wo8", [C, C], FP8, kind="ExternalInput")
    w1 = nc.dram_tensor("w1", [C, F], FP8, kind="ExternalInput")
    w2 = nc.dram_tensor("w2", [F, C], FP8, kind="ExternalInput")
    gb = nc.dram_tensor("gb", [6, C], F32R, kind="ExternalInput")
    bf1 = nc.dram_tensor("bf1", [F], F32, kind="ExternalInput")
    ones_in = nc.dram_tensor("ones_in", [1, P], F32R, kind="ExternalInput")
    outT = nc.dram_tensor("outT", [C, TQ], F32, kind="ExternalOutput")

    with tile.TileContext(nc) as tc, contextlib.ExitStack() as ctx:
        singles = ctx.enter_context(tc.tile_pool(name="singles", bufs=1))

        ones_fr = singles.tile([P, 1], F32R)
        nc.sync.dma_start(out=ones_fr, in_=ones_in[:, 0:1].to_broadcast([P, 1]))
        ones_bf = singles.tile([P, 1], BF16)
        nc.vector.memset(ones_bf, 1.0)
        eps_t = singles.tile([1, 1], F32)
        nc.vector.memset(eps_t, EPS)
        log32_t = singles.tile([P, 1], F32)
        nc.vector.memset(log32_t, LOG32)

        g_rows = singles.tile([1, 2, C], BF16)
        nc.gpsimd.dma_start(out=g_rows[:, 0, :], in_=gb[None, 0, :].bitcast(F32))
        nc.gpsimd.dma_start(out=g_rows[:, 1, :], in_=gb[None, 2, :].bitcast(F32))
        b1_pc = singles.tile([P, CB], F32)
        nc.sync.dma_start(out=b1_pc, in_=gb[1, :].rearrange("(k p) -> p k", p=P).bitcast(F32))
        b2_pc = singles.tile([P, CB], F32)
        nc.sync.dma_start(out=b2_pc, in_=gb[3, :].rearrange("(k p) -> p k", p=P).bitcast(F32))
        bo_pc = singles.tile([P, CB], F32)
        nc.sync.dma_start(out=bo_pc, in_=gb[4, :].rearrange("(k p) -> p k", p=P).bitcast(F32))
        bf2_pc = singles.tile([P, CB], F32)
        nc.sync.dma_start(out=bf2_pc, in_=gb[5, :].rearrange("(k p) -> p k", p=P).bitcast(F32))
        bf1_pc = singles.tile([P, FB], F32)
        nc.sync.dma_start(out=bf1_pc, in_=bf1[:].rearrange("(k p) -> p k", p=P))
        mask_sb = singles.tile([P, NSB, P], FP8)
        nc.sync.dma_start(out=mask_sb, in_=maskT[:, :, :])

        # --- persistent activation tiles (alloc order = reverse free order) ---
        xpbo, _f_xpbo = tc.tile([P, CB, TQ], BF16, name="xpbo")  # x + bo (own)
        qT, _f_qT = tc.tile([P, CB, TQ], FP8, name="qT")         # 32*q
        kT, _f_kT = tc.tile([P, CB, TKV], FP8, name="kT")        # 32*k
        v_aug, _f_v = tc.tile([P, NSB, H, HD + 1], FP8, name="v_aug")
        nc.vector.memset(v_aug[:, :, :, HD], 1.0)
        _att = [tc.tile([HD, H // 2, TQ], FP8, name=f"attn8u{u}")
                for u in range(2)]
        attn8u = [t for t, _ in _att]
        attn128, _f_a128 = tc.tile([P, H // 2, TQ], FP8, name="attn128")
        h8, free_h8 = tc.tile([P, CB, TKV], FP8, name="h8")
        w8pool = contextlib.ExitStack()
        w8p = w8pool.enter_context(tc.tile_pool(name="w8p", bufs=1))
        wq_sb = w8p.tile([P, CB, C], FP8, name="wq_sb")
        wk_sb = w8p.tile([P, CB, C], FP8, name="wk_sb")
        wv_sb = w8p.tile([P, CB, C], FP8, name="wv_sb")
        x_sb, free_x = tc.tile([P, CB, TKV], BF16, name="x_sb")

        for t0 in range(TKV // TQ):
            for cb in range(CB):
                tsl = slice(t0 * TQ, (t0 + 1) * TQ)
                nc.sync.dma_start(
                    out=x_sb[:, cb, tsl],
                    in_=xT[:, :].rearrange("(k p) t -> p k t", p=P)[:, cb, tsl])
        nc.sync.dma_start(out=wq_sb, in_=wq8[:, :].rearrange("(k p) n -> p k n", p=P))
        nc.sync.dma_start(out=wk_sb, in_=wk8[:, :].rearrange("(k p) n -> p k n", p=P))
        nc.sync.dma_start(out=wv_sb, in_=wv8[:, :].rearrange("(k p) n -> p k n", p=P))

        # xpbo = x + bo on own columns (feeds the z residual later)
        for cb in range(CB):
            nc.gpsimd.tensor_scalar_add(xpbo[:, cb, :], x_sb[:, cb, 0:TQ],
                                        bo_pc[:, cb : cb + 1])

        # ================= LN1 over all TKV tokens -> h8 (fp8) =================
        NCH = TKV // TQ  # 4 chunks of 512
        with contextlib.ExitStack() as lnc:
            stat_ps = lnc.enter_context(tc.tile_pool(name="ln_stat", bufs=2, space="PSUM"))
            ap_ps = lnc.enter_context(tc.tile_pool(name="ln_ap", bufs=1, space="PSUM"))
            lns = lnc.enter_context(tc.tile_pool(name="ln_sq", bufs=3))
            lnr = lnc.enter_context(tc.tile_pool(name="ln_rows", bufs=2))
            tmp_p = lnc.enter_context(tc.tile_pool(name="ln_tmp", bufs=2))

            m_tiles, s_tiles, r_tiles = {}, {}, {}

            def ln_means(t0, src, ntok):
                sl = slice(t0 * ntok, (t0 + 1) * ntok)
                m_ps = stat_ps.tile([1, ntok], F32, name="m_ps")
                for cb in range(CB):
                    nc.tensor.matmul(m_ps, ones_bf, src[:, cb, sl],
                                     start=(cb == 0), stop=(cb == CB - 1))
                m_tiles[t0] = m_ps

            def ln_sqs(t0, src, ntok, sq_dt=BF16, ones=None):
                sl = slice(t0 * ntok, (t0 + 1) * ntok)
                s_ps = stat_ps.tile([1, ntok], F32, name="s_ps")
                for cb in range(CB):
                    sq = lns.tile([P, ntok], F32R, name="sq")
                    if cb % 2 == 0:
                        nc.gpsimd.tensor_mul(sq, src[:, cb, sl], src[:, cb, sl])
                    else:
                        nc.scalar.activation(sq, src[:, cb, sl],
                                             mybir.ActivationFunctionType.Square)
                    nc.tensor.matmul(s_ps, ones_fr, sq,
                                     start=(cb == 0), stop=(cb == CB - 1))
                s_tiles[t0] = s_ps

            def ln_stats(t0, ntok):
                m_ps, s_ps = m_tiles.pop(t0), s_tiles.pop(t0)
                m_sb = lnr.tile([1, ntok], F32, name="m_sb")
                nc.scalar.mul(m_sb, m_ps, 1.0 / C)
                var = lnr.tile([1, ntok], F32, name="var")
                nc.scalar.mul(var, s_ps, 1.0 / C)
                msq = lnr.tile([1, ntok], F32, name="msq")
                nc.vector.tensor_mul(msq, m_sb, m_sb)
                nc.vector.tensor_sub(var, var, msq)
                nc.scalar.activation(var, var, mybir.ActivationFunctionType.Sqrt,
                                     bias=eps_t)
                rstd = lnr.tile([1, ntok], BF16, name="rstd")
                with nc.allow_low_precision(reason="f32r rstd"):
                    nc.vector.reciprocal(rstd, var)
                nm = lnr.tile([1, ntok], BF16, name="nm")
                nc.vector.tensor_mul(nm, m_sb, rstd)
                nc.scalar.mul(nm, nm, -1.0)
                r_tiles[t0] = (rstd, nm)

            def ln_apply(t0, src, dst, ntok, g_row, b_pc):
                sl = slice(t0 * ntok, (t0 + 1) * ntok)
                rstd, nm = r_tiles.pop(t0)
                sc_ps = ap_ps.tile([P, ntok], F32, name="sc_ps")
                bi_ps = ap_ps.tile([P, ntok], F32, name="bi_ps")
                for cb in range(CB):
                    csl = slice(cb * P, (cb + 1) * P)
                    nc.tensor.matmul(sc_ps, g_row[:, csl], rstd, start=True, stop=True)
                    nc.tensor.matmul(bi_ps, g_row[:, csl], nm, start=True, stop=True)
                    tmp = tmp_p.tile([P, ntok], F32R, name=f"tmp{cb % 2}")
                    nc.vector.tensor_mul(tmp, src[:, cb, sl], sc_ps)
                    nc.vector.scalar_tensor_tensor(
                        out=dst[:, cb, sl], in0=tmp,
                        scalar=b_pc[:, cb : cb + 1], in1=bi_ps,
                        op0=ADD, op1=ADD)

            pps = lnc.enter_context(tc.tile_pool(name="qkv_ps", bufs=2, space="PSUM"))

            def q_proj():
                for mb in range(CB):
                    ps = pps.tile([P, 2, 256], F32, name="ps_qkv")
                    for f in range(2):
                        fsl = slice(f * 256, (f + 1) * 256)
                        for kp in range(KP):
                            ksl = slice(2 * kp, 2 * kp + 2)
                            nc.tensor.matmul(
                                ps[:, f, :], wq_sb[:, ksl, mb * P : (mb + 1) * P],
                                h8[:, ksl, fsl],
                                start=(kp == 0), stop=(kp == KP - 1), perf_mode=DR)
                    nc.scalar.mul(qT[:, mb, :], ps.rearrange("p f n -> p (f n)"), 1.0)

            def k_chunk(t0):
                for mb in range(CB):
                    ps = pps.tile([P, 2, 256], F32, name="ps_qkv")
                    for f in range(2):
                        fsl = slice(t0 * TQ + f * 256, t0 * TQ + (f + 1) * 256)
                        for kp in range(KP):
                            ksl = slice(2 * kp, 2 * kp + 2)
                            nc.tensor.matmul(
                                ps[:, f, :], wk_sb[:, ksl, mb * P : (mb + 1) * P],
                                h8[:, ksl, fsl],
                                start=(kp == 0), stop=(kp == KP - 1), perf_mode=DR)
                    out_sl = kT[:, mb, t0 * TQ : (t0 + 1) * TQ]
                    if mb % 2 == 0:
                        nc.scalar.mul(out_sl, ps.rearrange("p f n -> p (f n)"), 1.0)
                    else:
                        nc.vector.tensor_copy(out_sl, ps.rearrange("p f n -> p (f n)"))

            def v_chunk(t0):
                for ti in range(4):
                    tb = 4 * t0 + ti
                    for hh in range(2):
                        ps = pps.tile([P, 2, 256], F32, name="ps_qkv")
                        for f in range(2):
                            fsl = slice(hh * 512 + f * 256, hh * 512 + (f + 1) * 256)
                            for kp in range(KP):
                                ksl = slice(2 * kp, 2 * kp + 2)
                                nc.tensor.matmul(
                                    ps[:, f, :],
                                    h8[:, ksl, tb * P : (tb + 1) * P],
                                    wv_sb[:, ksl, fsl],
                                    start=(kp == 0), stop=(kp == KP - 1), perf_mode=DR)
                        dst = v_aug[:, tb, hh * 8 : (hh + 1) * 8, 0:HD]
                        srcp = ps.rearrange("p f (h d) -> p (f h) d", d=HD)
                        if tb % 2 == 0:
                            nc.vector.tensor_copy(dst, srcp)
                        else:
                            nc.scalar.mul(dst, srcp, 1.0)

            g1_row = g_rows[:, 0, :]
            # per-chunk LN1 -> QKV fusion keeps PE fed while DVE applies
            ln_means(0, x_sb, TQ)
            ln_sqs(0, x_sb, TQ)
            ln_means(1, x_sb, TQ)
            ln_stats(0, TQ)
            ln_sqs(1, x_sb, TQ)
            ln_apply(0, x_sb, h8, TQ, g1_row, b1_pc)
            q_proj()
            k_chunk(0)
            v_chunk(0)
            ln_means(2, x_sb, TQ)
            ln_stats(1, TQ)
            ln_sqs(2, x_sb, TQ)
            ln_apply(1, x_sb, h8, TQ, g1_row, b1_pc)
            k_chunk(1)
            v_chunk(1)
            ln_means(3, x_sb, TQ)
            ln_stats(2, TQ)
            ln_sqs(3, x_sb, TQ)
            ln_apply(2, x_sb, h8, TQ, g1_row, b1_pc)
            k_chunk(2)
            v_chunk(2)
            ln_stats(3, TQ)
            ln_apply(3, x_sb, h8, TQ, g1_row, b1_pc)
            k_chunk(3)
            v_chunk(3)

        free_x()
        w8pool.close()
        free_h8()

        zT, _f_zT = tc.tile([P, CB, TQ], F32R, name="zT")
        oAz, _f_oAz = tc.tile([P, CB, TQ], F32R, name="oAz")
        h2T, _f_h2 = tc.tile([P, CB, TQ], BF16, name="h2T")
        h2q8, _f_h2q = tc.tile([P, CB, TQ], FP8, name="h2q8")
        h2r8, _f_h2r = tc.tile([P, CB, TQ], FP8, name="h2r8")
        aT8, _f_a8 = tc.tile([P, FB, TQ], FP8, name="aT8")
        aTr8, _f_ar8 = tc.tile([P, FB, TQ], FP8, name="aTr8")

        # prefetch wo and first FFN1 weights during attention
        # (w1c opens first: pools are a LIFO stack and wo_p closes earlier)
        w1pool = contextlib.ExitStack()
        w1c = w1pool.enter_context(tc.tile_pool(name="w1c", bufs=2))
        wo_pool = contextlib.ExitStack()
        wo_sb = wo_pool.enter_context(tc.tile_pool(name="wo_p", bufs=1)).tile(
            [P, CB, C], FP8, name="wo_sb")
        nc.sync.dma_start(out=wo_sb, in_=wo8[:, :].rearrange("(k p) n -> p k n", p=P))

        # ================= attention (per head pair) =================
        with contextlib.ExitStack() as p3:
            sc_ps_pool = p3.enter_context(tc.tile_pool(name="sc_ps", bufs=2, space="PSUM"))
            pair_ps_pool = p3.enter_context(tc.tile_pool(name="pair_ps", bufs=2, space="PSUM"))
            bc_pool = p3.enter_context(tc.tile_pool(name="bc", bufs=2))
            probs_pool = p3.enter_context(tc.tile_pool(name="probs", bufs=2))
            rec_pool = p3.enter_context(tc.tile_pool(name="rec", bufs=2))

            # key-block pairs (2-bank batched exp; boundary overcompute is
            # masked/unread)
            SBG = [[0, 1], [2, 3], [4, 5], [6, 7], [8, 9], [10, 11],
                   [12, 13], [14, 15]]
            for pair in range(H // 2):
                probs8 = probs_pool.tile([P, 2, NSB, TQ], FP8, name="probs8")
                ps_h = [pair_ps_pool.tile([HD + 1, TQ], F32, name=f"ps_h{u}")
                        for u in range(2)]
                for grp in SBG:
                    q0 = QL[grp[0]]
                    n = len(grp)
                    for u in range(2):
                        prow = slice(u * HD, (u + 1) * HD)
                        ps3 = sc_ps_pool.tile([P, 2, TQ], F32, name="ps_s")
                        for i, sb in enumerate(grp):
                            qi = QL[sb]
                            nc.tensor.matmul(
                                ps3[:, i, qi:TQ],
                                kT[prow, pair, sb * P : (sb + 1) * P],
                                qT[prow, pair, qi:TQ],
                                start=True, stop=True)
                        nc.scalar.activation(
                            probs8[:, u, grp[0] : grp[0] + n, q0:TQ],
                            ps3[:, 0:n, q0:TQ],
                            mybir.ActivationFunctionType.Exp,
                            scale=SCALE2, bias=log32_t)
                        for sb in grp:
                            qb = QL[sb]
                            eng = nc.gpsimd if sb % 3 == 2 else nc.vector
                            eng.tensor_mul(
                                probs8[:, u, sb, qb : qb + P],
                                probs8[:, u, sb, qb : qb + P],
                                mask_sb[:, sb, :])
                for sb in range(NSB):
                    first, last = (sb == 0), (sb == NSB - 1)
                    for u in range(2):
                        nc.tensor.matmul(
                            ps_h[u][:, QL[sb]:TQ],
                            v_aug[:, sb, 2 * pair + u, :],
                            probs8[:, u, sb, QL[sb]:TQ],
                            start=first, stop=last)
                rec_pair = rec_pool.tile([1, 2, TQ], BF16, name="rec_pair")
                for u in range(2):
                    with nc.allow_low_precision(reason="softmax denom"):
                        nc.vector.reciprocal(rec_pair[:, u, :],
                                             ps_h[u][HD : HD + 1, :])
                bc_sb = bc_pool.tile([HD, 2, TQ], BF16, name="bc_sb")
                nc.gpsimd.partition_broadcast(bc_sb, rec_pair)
                for u in range(2):
                    nc.vector.tensor_mul(attn8u[u][:, pair, :],
                                         ps_h[u][0:HD, :], bc_sb[:, u, :])
                # stack this pair onto 128 partitions early (SBUF->SBUF DMA)
                nc.sync.dma_start(out=attn128[0:HD, pair, :],
                                  in_=attn8u[0][:, pair, :])
                nc.sync.dma_start(out=attn128[HD:P, pair, :],
                                  in_=attn8u[1][:, pair, :])

        # ========== wo (fp8 DR) + residual -> zT, LN2 fused into the loop ==========
        with contextlib.ExitStack() as p4:
            stat_ps = p4.enter_context(tc.tile_pool(name="l2_stat", bufs=1, space="PSUM"))
            ap_ps = p4.enter_context(tc.tile_pool(name="l2_ap", bufs=1, space="PSUM"))
            lns = p4.enter_context(tc.tile_pool(name="l2_sq", bufs=3))
            lnr = p4.enter_context(tc.tile_pool(name="l2_rows", bufs=1))
            tmp_p = p4.enter_context(tc.tile_pool(name="l2_tmp", bufs=2))
            ab_p = p4.enter_context(tc.tile_pool(name="ab", bufs=2))
            wo_scope = contextlib.ExitStack()
            ops = wo_scope.enter_context(tc.tile_pool(name="wo_ps", bufs=2, space="PSUM"))

            m_ps = stat_ps.tile([1, TQ], F32, name="m_ps")
            s_ps = stat_ps.tile([1, TQ], F32, name="s_ps")

            def wo_mb(mb):
                ps = ops.tile([P, 2, 256], F32, name="ps_z")
                for f in range(2):
                    fsl = slice(f * 256, (f + 1) * 256)
                    for kp in range(KP):
                        ksl = slice(2 * kp, 2 * kp + 2)
                        nc.tensor.matmul(
                            ps[:, f, :], wo_sb[:, ksl, mb * P : (mb + 1) * P],
                            attn128[:, ksl, fsl],
                            start=(kp == 0), stop=(kp == KP - 1), perf_mode=DR)
                nc.vector.scalar_tensor_tensor(
                    out=zT[:, mb, :], in0=ps.rearrange("p f n -> p (f n)"),
                    scalar=1.0 / (WS * WS), in1=xpbo[:, mb, :],
                    op0=MUL, op1=ADD)

            def l2_mean(cb):
                nc.tensor.matmul(m_ps, ones_fr, zT[:, cb, :],
                                 start=(cb == 0), stop=(cb == CB - 1),
                                 skip_group_check=True)

            def l2_sq(cb):
                sq = lns.tile([P, TQ], F32R, name="sq")
                nc.scalar.activation(sq, zT[:, cb, :],
                                     mybir.ActivationFunctionType.Square)
                nc.tensor.matmul(s_ps, ones_fr, sq,
                                 start=(cb == 0), stop=(cb == CB - 1),
                                 skip_group_check=True)

            for mb in range(CB):
                wo_mb(mb)
                if mb >= 1:
                    l2_mean(mb - 1)
                if mb >= 2:
                    l2_sq(mb - 2)
            l2_mean(CB - 1)
            l2_sq(CB - 2)
            l2_sq(CB - 1)
            wo_scope.close()
            fps = p4.enter_context(tc.tile_pool(name="ffn_ps", bufs=2, space="PSUM"))
            fps2 = p4.enter_context(tc.tile_pool(name="ffn2_ps", bufs=2, space="PSUM"))
            w2c = p4.enter_context(tc.tile_pool(name="w2c", bufs=2))
            outp = p4.enter_context(tc.tile_pool(name="outp", bufs=2))

            def ffn2_pass(half, src_acc, dst_write):
                k0 = half * (FB // 2)
                for mb in range(CB):
                    w2_c = w2c.tile([P, FB // 2, P], FP8, name="w2_c", bufs=2)
                    nc.sync.dma_start(
                        out=w2_c,
                        in_=w2[:, :].rearrange("(k p) n -> p k n", p=P)[
                            :, k0 : k0 + FB // 2, mb * P : (mb + 1) * P])
                    ps = fps2.tile([P, 2, 256], F32, name="ps_o")
                    for t0 in range(2):
                        tsl = slice(t0 * 256, (t0 + 1) * 256)
                        for ti, src8 in enumerate((aT8, aTr8)):
                            for kp in range(FB // 4):
                                ksl = slice(2 * kp, 2 * kp + 2)
                                nc.tensor.matmul(
                                    ps[:, t0, :], w2_c[:, ksl, :],
                                    src8[:, k0 + 2 * kp : k0 + 2 * kp + 2, tsl],
                                    start=(ti == 0 and kp == 0),
                                    stop=(ti == 1 and kp == FB // 4 - 1),
                                    perf_mode=DR)
                    dst_write(mb, ps)

            m_sb = lnr.tile([1, TQ], F32, name="m_sb")
            nc.scalar.mul(m_sb, m_ps, 1.0 / C)
            var = lnr.tile([1, TQ], F32, name="var")
            nc.scalar.mul(var, s_ps, 1.0 / C)
            msq = lnr.tile([1, TQ], F32, name="msq")
            nc.vector.tensor_mul(msq, m_sb, m_sb)
            nc.vector.tensor_sub(var, var, msq)
            nc.scalar.activation(var, var, mybir.ActivationFunctionType.Sqrt,
                                 bias=eps_t)
            rstd = lnr.tile([1, TQ], BF16, name="rstd")
            with nc.allow_low_precision(reason="f32r rstd"):
                nc.vector.reciprocal(rstd, var)
            nm = lnr.tile([1, TQ], BF16, name="nm")
            nc.vector.tensor_mul(nm, m_sb, rstd)
            nc.scalar.mul(nm, nm, -1.0)

            g_row = g_rows[:, 1, :]
            sc_ps = ap_ps.tile([P, TQ], F32, name="sc_ps")
            bi_ps = ap_ps.tile([P, TQ], F32, name="bi_ps")
            for cb in range(CB):
                csl = slice(cb * P, (cb + 1) * P)
                nc.tensor.matmul(sc_ps, g_row[:, csl], rstd, start=True, stop=True)
                nc.tensor.matmul(bi_ps, g_row[:, csl], nm, start=True, stop=True)
                tmp = tmp_p.tile([P, TQ], F32R, name="tmp")
                nc.vector.tensor_mul(tmp, zT[:, cb, :], sc_ps)
                nc.vector.scalar_tensor_tensor(
                    out=h2T[:, cb, :], in0=tmp,
                    scalar=b2_pc[:, cb : cb + 1], in1=bi_ps,
                    op0=ADD, op1=ADD)
                nc.scalar.mul(h2q8[:, cb, :], h2T[:, cb, :], 1.0)
                nc.gpsimd.tensor_sub(h2r8[:, cb, :], h2T[:, cb, :],
                                       h2q8[:, cb, :])
                # fold bf2 into zT now that LN2 is done with it
                nc.gpsimd.tensor_scalar_add(zT[:, cb, :], zT[:, cb, :],
                                            bf2_pc[:, cb : cb + 1])

            # ---------------- FFN1: (h2q8 + h2r8) @ w1 (fp8 DR) ----------------
            def ffn1_wg(wg):
                wtile = w1c.tile([P, CB, 4 * P], FP8, name="w1_c", bufs=2)
                nc.sync.dma_start(
                    out=wtile,
                    in_=w1[:, :].rearrange("(k p) n -> p k n", p=P)[
                        :, :, wg * 4 * P : (wg + 1) * 4 * P])
                for fi in range(4):
                    fb = wg * 4 + fi
                    ps = fps.tile([P, 2, 256], F32, name="ps_a")
                    for t0 in range(2):
                        tsl = slice(t0 * 256, (t0 + 1) * 256)
                        for ti, src8 in enumerate((h2q8, h2r8)):
                            for kp in range(KP):
                                ksl = slice(2 * kp, 2 * kp + 2)
                                nc.tensor.matmul(
                                    ps[:, t0, :],
                                    wtile[:, ksl, fi * P : (fi + 1) * P],
                                    src8[:, ksl, tsl],
                                    start=(ti == 0 and kp == 0),
                                    stop=(ti == 1 and kp == KP - 1),
                                    perf_mode=DR)
                    aTb = ab_p.tile([P, TQ], BF16, name="aTb")
                    nc.scalar.activation(aTb, ps.rearrange("p f n -> p (f n)"),
                                         mybir.ActivationFunctionType.Relu,
                                         scale=1.0 / WS,
                                         bias=bf1_pc[:, fb : fb + 1])
                    nc.vector.tensor_copy(aT8[:, fb, :], aTb)
                    nc.gpsimd.tensor_sub(aTr8[:, fb, :], aTb, aT8[:, fb, :])

            def wA(mb, ps):
                nc.vector.scalar_tensor_tensor(
                    out=oAz[:, mb, :], in0=ps.rearrange("p f n -> p (f n)"),
                    scalar=1.0 / WS2, in1=zT[:, mb, :], op0=MUL, op1=ADD)

            def wB(mb, ps):
                o_sb = outp.tile([P, TQ], F32, name="o_sb")
                nc.vector.scalar_tensor_tensor(
                    out=o_sb, in0=ps.rearrange("p f n -> p (f n)"),
                    scalar=1.0 / WS2, in1=oAz[:, mb, :], op0=MUL, op1=ADD)
                nc.sync.dma_start(
                    out=outT[:, :].rearrange("(k p) t -> p k t", p=P)[:, mb, :],
                    in_=o_sb)

            for wg in range(4):
                ffn1_wg(wg)
            ffn2_pass(0, None, wA)
            for wg in range(4, 8):
                ffn1_wg(wg)
            ffn2_pass(1, None, wB)
        wo_pool.close()
        w1pool.close()

        # release singleton tiles in LIFO order
        _f_ar8()
        _f_a8()
        _f_h2r()
        _f_h2q()
        _f_h2()
        _f_oAz()
        _f_zT()
        _f_a128()
        _att[1][1]()
        _att[0][1]()
        _f_v()
        _f_kT()
        _f_qT()
        _f_xpbo()
    nc.compile()
    return nc


_CACHE = {}


def _get_built():
    if "nc" not in _CACHE:
        _CACHE["nc"] = build_kernel()
    return _CACHE["nc"]


def _qidx(j):
    return np.concatenate([np.arange((4 * i + j) * P, (4 * i + j + 1) * P)
                           for i in range(4)])


def _perm_times(j):
    own = [4 * i + j for i in range(4)]
    other = sorted(set(range(NSB)) - set(own))
    return own + other


def _build_in_maps(x, wq, wk, wv, wo, bo, g1, b1, g2, b2, w1, bf1, w2, bf2):
    f = np.float32
    bf = ml_dtypes.bfloat16
    f8 = ml_dtypes.float8_e4m3
    x = np.asarray(x, f)
    wq_m = (np.asarray(wq, f).transpose(1, 0, 2).reshape(C, C) * WS).astype(f8)
    wk_m = (np.asarray(wk, f).transpose(1, 0, 2).reshape(C, C) * WS).astype(f8)
    wv_m = (np.asarray(wv, f).transpose(1, 0, 2).reshape(C, C) * WS).astype(f8)
    wo_m = (np.asarray(wo, f) * WS).astype(f8)
    w1_m = (np.asarray(w1, f) * WS).astype(f8)
    w2_m = (np.asarray(w2, f) * WS2).astype(f8)
    gb_m = np.ascontiguousarray(np.stack([np.asarray(a, f) for a in
                                          (g1, b1, g2, b2, bo, bf2)]))
    bf1_m = np.ascontiguousarray(np.asarray(bf1, f))

    in_maps = []
    for c in range(8):
        b, j = divmod(c, 4)
        ptimes = _perm_times(j)
        tok = np.concatenate([np.arange(t * P, (t + 1) * P) for t in ptimes])
        xT = np.ascontiguousarray(x[b].T[:, tok]).astype(bf)
        # boundary mask per key block kb: q block QL[kb]//P vs key time
        pp = np.arange(P)[:, None]
        cc = np.arange(P)[None, :]
        maskT = np.empty((P, NSB, P), f)
        for kb in range(NSB):
            qt = 4 * (QL[kb] // P) + j
            kt = ptimes[kb]
            maskT[:, kb, :] = ((qt - kt) * P + cc >= pp).astype(f)
        in_maps.append({
            "xT": xT, "maskT": maskT.astype(f8),
            "wq8": wq_m, "wk8": wk_m, "wv8": wv_m, "wo8": wo_m,
            "w1": w1_m, "w2": w2_m, "gb": gb_m, "bf1": bf1_m,
            "ones_in": np.ones((1, P), np.float32),
        })
    return in_maps


def _gather(results):
    out = np.empty((B, T, C), np.float32)
    for c in range(8):
        b, j = divmod(c, 4)
        out[b, _qidx(j)] = results[c]["outT"].T
    return out


def kernel(**inputs):
    in_maps = _build_in_maps(**inputs)
    nc = _get_built()
    res = run_bass_kernel_spmd(nc, in_maps, core_ids=list(range(8)))
    return _gather(res.results)


def run_traced(**inputs):
    in_maps = _build_in_maps(**inputs)
    nc = _get_built()
    return run_bass_kernel_spmd(nc, in_maps, core_ids=list(range(8)), trace=True)


# revision 30
# speedup vs baseline: 2.0788x; 1.0424x over previous
"""Trainium2 Bass kernel for a dense transformer decoder block (v2).

Sharding: pure data-parallel over 8 cores. Core c=(b*4+j) owns batch b and
query blocks {4i+j : i=0..3}. Host PERMUTES each core's 2048 tokens so the
core's own 512 query tokens come first; causality is enforced by per-core
per-key-block boundary masks, so the device program is identical on all
cores (j only affects host-prepared data).

Numerics (rms rel err budget ~2.3e-3 vs 2e-2 gate, verified by host emu):
- x, K/Q in bf16; probs/V/attn in fp8e4 (attention output is ~1.5% of the
  residual stream here: scores carry C**-0.5 scaling so softmax is nearly
  uniform -> fp8 noise in the attention path is negligible downstream).
- Q/K/V and wo projections run fp8e4 with DoubleRow perf mode (2 x 128-deep
  contraction per matmul at 0.5 cycles/row). Weights host-scaled x32; the
  scale folds into the softmax scale (1/1024), cancels in the softmax
  normalize, and is divided out in the z epilogue (1/1024).
- FFN stays bf16 (fp8 there costs ~1e-2 rms - too close to the gate).

Cost-model facts this build exploits (instruction_cost_v2.rs):
- matmul time = out_free_size * pe_cycle * cpr; cpr: bf16/f32r(>=256 free)=1.0,
  fp8 DoubleRow=0.5 (with 256-deep contraction -> 4x bf16 FLOP rate).
- PE p-state ramps with *continuous* busy time (low 1.54ns/row after idle,
  0.83 mid, 0.42 full after 3us) -> LN statistic matmuls are emitted batched
  and interleaved so the PE never sleeps between them.
- Collectives cost 15us fixed + 40GB/s -> no collectives; duplicated
  K/V projection is cheap in fp8-DoubleRow instead.
"""

import contextlib

import numpy as np
import ml_dtypes

import concourse.bass as bass
import concourse.bacc as bacc
import concourse.mybir as mybir
import concourse.tile as tile
from concourse.bass_utils import run_bass_kernel_spmd

B, T, C, H, HD, F = 2, 2048, 1024, 16, 64, 4096
EPS = 1e-5
P = 128
CB = C // P          # 8 emb chunks
KP = CB // 2         # 4 DoubleRow chunk-pairs
FB = F // P          # 32 ffn chunks
TQ = 512             # own query tokens per core
TKV = 2048           # kv tokens (full batch, permuted: own 512 first)
NSB = TKV // P       # 16 key blocks
WS = 32.0            # host weight scale for fp8 (wq/wk/wv/wo/w1)
WS2 = 64.0           # host weight scale for fp8 w2
SCALE2 = float(C) ** -0.5 / (WS * WS)   # folds both x32 into softmax scale
LOG32 = float(np.log(32.0))             # probs8 = 32*exp(score): fp8 range

F32 = mybir.dt.float32
F32R = mybir.dt.float32r
BF16 = mybir.dt.bfloat16
FP8 = mybir.dt.float8e4
DR = mybir.MatmulPerfMode.DoubleRow
ADD = mybir.AluOpType.add
MUL = mybir.AluOpType.mult

# q_lo per key block kb (j-independent under the own-first permutation):
# own blocks kb<4 sit at q block kb; non-own block n=kb-4 needs q blocks
# i >= n//3 (boundary block handled by mask).
QL = [kb * P if kb < 4 else ((kb - 4) // 3) * P for kb in range(NSB)]


def build_kernel():
    nc = bacc.Bacc("TRN2", num_devices=8)

    xT = nc.dram_tensor("xT", [C, TKV], BF16, kind="ExternalInput")
    maskT = nc.dram_tensor("maskT", [P, NSB, P], FP8, kind="ExternalInput")
    wq8 = nc.dram_tensor("wq8", [C, C], FP8, kind="ExternalInput")
    wk8 = nc.dram_tensor("wk8", [C, C], FP8, kind="ExternalInput")
    wv8 = nc.dram_tensor("wv8", [C, C], FP8, kind="ExternalInput")
    wo8 = nc.dram_tensor("wo8", [C, C], FP8, kind="ExternalInput")
    w1 = nc.dram_tensor("w1", [C, F], FP8, kind="ExternalInput")
    w2 = nc.dram_tensor("w2", [F, C], FP8, kind="ExternalInput")
    gb = nc.dram_tensor("gb", [6, C], F32R, kind="ExternalInput")
    bf1 = nc.dram_tensor("bf1", [F], F32, kind="ExternalInput")
    ones_in = nc.dram_tensor("ones_in", [1, P], F32R, kind="ExternalInput")
    outT = nc.dram_tensor("outT", [C, TQ], F32, kind="ExternalOutput")

    with tile.TileContext(nc) as tc, contextlib.ExitStack() as ctx:
        singles = ctx.enter_context(tc.tile_pool(name="singles", bufs=1))

        ones_fr = singles.tile([P, 1], F32R)
        nc.sync.dma_start(out=ones_fr, in_=ones_in[:, 0:1].to_broadcast([P, 1]))
        ones_bf = singles.tile([P, 1], BF16)
        nc.vector.memset(ones_bf, 1.0)
        eps_t = singles.tile([1, 1], F32)
        nc.vector.memset(eps_t, EPS)
        log32_t = singles.tile([P, 1], F32)
        nc.vector.memset(log32_t, LOG32)

        g_rows = singles.tile([1, 2, C], BF16)
        nc.gpsimd.dma_start(out=g_rows[:, 0, :], in_=gb[None, 0, :].bitcast(F32))
        nc.gpsimd.dma_start(out=g_rows[:, 1, :], in_=gb[None, 2, :].bitcast(F32))
        b1_pc = singles.tile([P, CB], F32)
        nc.sync.dma_start(out=b1_pc, in_=gb[1, :].rearrange("(k p) -> p k", p=P).bitcast(F32))
        b2_pc = singles.tile([P, CB], F32)
        nc.sync.dma_start(out=b2_pc, in_=gb[3, :].rearrange("(k p) -> p k", p=P).bitcast(F32))
        bo_pc = singles.tile([P, CB], F32)
        nc.sync.dma_start(out=bo_pc, in_=gb[4, :].rearrange("(k p) -> p k", p=P).bitcast(F32))
        bf2_pc = singles.tile([P, CB], F32)
        nc.sync.dma_start(out=bf2_pc, in_=gb[5, :].rearrange("(k p) -> p k", p=P).bitcast(F32))
        bf1_pc = singles.tile([P, FB], F32)
        nc.sync.dma_start(out=bf1_pc, in_=bf1[:].rearrange("(k p) -> p k", p=P))
        mask_sb = singles.tile([P, NSB, P], FP8)
        nc.sync.dma_start(out=mask_sb, in_=maskT[:, :, :])

        # --- persistent activation tiles (alloc order = reverse free order) ---
        xpbo, _f_xpbo = tc.tile([P, CB, TQ], BF16, name="xpbo")  # x + bo (own)
        qT, _f_qT = tc.tile([P, CB, TQ], FP8, name="qT")         # 32*q
        kT, _f_kT = tc.tile([P, CB, TKV], FP8, name="kT")        # 32*k
        v_aug, _f_v = tc.tile([P, NSB, H, HD + 1], FP8, name="v_aug")
        nc.vector.memset(v_aug[:, :, :, HD], 1.0)
        _att = [tc.tile([HD, H // 2, TQ], FP8, name=f"attn8u{u}")
                for u in range(2)]
        attn8u = [t for t, _ in _att]
        attn128, _f_a128 = tc.tile([P, H // 2, TQ], FP8, name="attn128")
        h8, free_h8 = tc.tile([P, CB, TKV], FP8, name="h8")
        w8pool = contextlib.ExitStack()
        w8p = w8pool.enter_context(tc.tile_pool(name="w8p", bufs=1))
        wq_sb = w8p.tile([P, CB, C], FP8, name="wq_sb")
        wk_sb = w8p.tile([P, CB, C], FP8, name="wk_sb")
        wv_sb = w8p.tile([P, CB, C], FP8, name="wv_sb")
        x_sb, free_x = tc.tile([P, CB, TKV], BF16, name="x_sb")

        for t0 in range(TKV // TQ):
            for cb in range(CB):
                tsl = slice(t0 * TQ, (t0 + 1) * TQ)
                nc.sync.dma_start(
                    out=x_sb[:, cb, tsl],
                    in_=xT[:, :].rearrange("(k p) t -> p k t", p=P)[:, cb, tsl])
        nc.sync.dma_start(out=wq_sb, in_=wq8[:, :].rearrange("(k p) n -> p k n", p=P))
        nc.sync.dma_start(out=wk_sb, in_=wk8[:, :].rearrange("(k p) n -> p k n", p=P))
        nc.sync.dma_start(out=wv_sb, in_=wv8[:, :].rearrange("(k p) n -> p k n", p=P))

        # xpbo = x + bo on own columns (feeds the z residual later)
        for cb in range(CB):
            nc.gpsimd.tensor_scalar_add(xpbo[:, cb, :], x_sb[:, cb, 0:TQ],
                                        bo_pc[:, cb : cb + 1])

        # ================= LN1 over all TKV tokens -> h8 (fp8) =================
        NCH = TKV // TQ  # 4 chunks of 512
        with contextlib.ExitStack() as lnc:
            stat_ps = lnc.enter_context(tc.tile_pool(name="ln_stat", bufs=2, space="PSUM"))
            ap_ps = lnc.enter_context(tc.tile_pool(name="ln_ap", bufs=1, space="PSUM"))
            lns = lnc.enter_context(tc.tile_pool(name="ln_sq", bufs=3))
            lnr = lnc.enter_context(tc.tile_pool(name="ln_rows", bufs=2))
            tmp_p = lnc.enter_context(tc.tile_pool(name="ln_tmp", bufs=2))

            m_tiles, s_tiles, r_tiles = {}, {}, {}

            def ln_means(t0, src, ntok):
                sl = slice(t0 * ntok, (t0 + 1) * ntok)
                m_ps = stat_ps.tile([1, ntok], F32, name="m_ps")
                for cb in range(CB):
                    nc.tensor.matmul(m_ps, ones_bf, src[:, cb, sl],
                                     start=(cb == 0), stop=(cb == CB - 1))
                m_tiles[t0] = m_ps

            def ln_sqs(t0, src, ntok, sq_dt=BF16, ones=None):
                sl = slice(t0 * ntok, (t0 + 1) * ntok)
                s_ps = stat_ps.tile([1, ntok], F32, name="s_ps")
                for cb in range(CB):
                    sq = lns.tile([P, ntok], F32R, name="sq")
                    if cb % 2 == 0:
                        nc.gpsimd.tensor_mul(sq, src[:, cb, sl], src[:, cb, sl])
                    else:
                        nc.scalar.activation(sq, src[:, cb, sl],
                                             mybir.ActivationFunctionType.Square)
                    nc.tensor.matmul(s_ps, ones_fr, sq,
                                     start=(cb == 0), stop=(cb == CB - 1))
                s_tiles[t0] = s_ps

            def ln_stats(t0, ntok):
                m_ps, s_ps = m_tiles.pop(t0), s_tiles.pop(t0)
                m_sb = lnr.tile([1, ntok], F32, name="m_sb")
                nc.scalar.mul(m_sb, m_ps, 1.0 / C)
                var = lnr.tile([1, ntok], F32, name="var")
                nc.scalar.mul(var, s_ps, 1.0 / C)
                msq = lnr.tile([1, ntok], F32, name="msq")
                nc.vector.tensor_mul(msq, m_sb, m_sb)
                nc.vector.tensor_sub(var, var, msq)
                nc.scalar.activation(var, var, mybir.ActivationFunctionType.Sqrt,
                                     bias=eps_t)
                rstd = lnr.tile([1, ntok], BF16, name="rstd")
                with nc.allow_low_precision(reason="f32r rstd"):
                    nc.vector.reciprocal(rstd, var)
                nm = lnr.tile([1, ntok], BF16, name="nm")
                nc.vector.tensor_mul(nm, m_sb, rstd)
                nc.scalar.mul(nm, nm, -1.0)
                r_tiles[t0] = (rstd, nm)

            def ln_apply(t0, src, dst, ntok, g_row, b_pc):
                sl = slice(t0 * ntok, (t0 + 1) * ntok)
                rstd, nm = r_tiles.pop(t0)
                sc_ps = ap_ps.tile([P, ntok], F32, name="sc_ps")
                bi_ps = ap_ps.tile([P, ntok], F32, name="bi_ps")
                for cb in range(CB):
                    csl = slice(cb * P, (cb + 1) * P)
                    nc.tensor.matmul(sc_ps, g_row[:, csl], rstd, start=True, stop=True)
                    nc.tensor.matmul(bi_ps, g_row[:, csl], nm, start=True, stop=True)
                    tmp = tmp_p.tile([P, ntok], F32R, name=f"tmp{cb % 2}")
                    nc.vector.tensor_mul(tmp, src[:, cb, sl], sc_ps)
                    nc.vector.scalar_tensor_tensor(
                        out=dst[:, cb, sl], in0=tmp,
                        scalar=b_pc[:, cb : cb + 1], in1=bi_ps,
                        op0=ADD, op1=ADD)

            pps = lnc.enter_context(tc.tile_pool(name="qkv_ps", bufs=2, space="PSUM"))

            def q_proj():
                for mb in range(CB):
                    ps = pps.tile([P, 2, 256], F32, name="ps_qkv")
                    for f in range(2):
                        fsl = slice(f * 256, (f + 1) * 256)
                        for kp in range(KP):
                            ksl = slice(2 * kp, 2 * kp + 2)
                            nc.tensor.matmul(
                                ps[:, f, :], wq_sb[:, ksl, mb * P : (mb + 1) * P],
                                h8[:, ksl, fsl],
                                start=(kp == 0), stop=(kp == KP - 1), perf_mode=DR)
                    nc.scalar.mul(qT[:, mb, :], ps.rearrange("p f n -> p (f n)"), 1.0)

            def k_chunk(t0):
                for mb in range(CB):
                    ps = pps.tile([P, 2, 256], F32, name="ps_qkv")
                    for f in range(2):
                        fsl = slice(t0 * TQ + f * 256, t0 * TQ + (f + 1) * 256)
                        for kp in range(KP):
                            ksl = slice(2 * kp, 2 * kp + 2)
                            nc.tensor.matmul(
                                ps[:, f, :], wk_sb[:, ksl, mb * P : (mb + 1) * P],
                                h8[:, ksl, fsl],
                                start=(kp == 0), stop=(kp == KP - 1), perf_mode=DR)
                    out_sl = kT[:, mb, t0 * TQ : (t0 + 1) * TQ]
                    if mb % 2 == 0:
                        nc.scalar.mul(out_sl, ps.rearrange("p f n -> p (f n)"), 1.0)
                    else:
                        nc.vector.tensor_copy(out_sl, ps.rearrange("p f n -> p (f n)"))

            def v_chunk(t0):
                for ti in range(4):
                    tb = 4 * t0 + ti
                    for hh in range(2):
                        ps = pps.tile([P, 2, 256], F32, name="ps_qkv")
                        for f in range(2):
                            fsl = slice(hh * 512 + f * 256, hh * 512 + (f + 1) * 256)
                            for kp in range(KP):
                                ksl = slice(2 * kp, 2 * kp + 2)
                                nc.tensor.matmul(
                                    ps[:, f, :],
                                    h8[:, ksl, tb * P : (tb + 1) * P],
                                    wv_sb[:, ksl, fsl],
                                    start=(kp == 0), stop=(kp == KP - 1), perf_mode=DR)
                        dst = v_aug[:, tb, hh * 8 : (hh + 1) * 8, 0:HD]
                        srcp = ps.rearrange("p f (h d) -> p (f h) d", d=HD)
                        if tb % 2 == 0:
                            nc.vector.tensor_copy(dst, srcp)
                        else:
                            nc.scalar.mul(dst, srcp, 1.0)

            g1_row = g_rows[:, 0, :]
            # per-chunk LN1 -> QKV fusion keeps PE fed while DVE applies
            ln_means(0, x_sb, TQ)
            ln_sqs(0, x_sb, TQ)
            ln_means(1, x_sb, TQ)
            ln_stats(0, TQ)
            ln_sqs(1, x_sb, TQ)
            ln_apply(0, x_sb, h8, TQ, g1_row, b1_pc)
            q_proj()
            k_chunk(0)
            v_chunk(0)
            ln_means(2, x_sb, TQ)
            ln_stats(1, TQ)
            ln_sqs(2, x_sb, TQ)
            ln_apply(1, x_sb, h8, TQ, g1_row, b1_pc)
            k_chunk(1)
            v_chunk(1)
            ln_means(3, x_sb, TQ)
            ln_stats(2, TQ)
            ln_sqs(3, x_sb, TQ)
            ln_apply(2, x_sb, h8, TQ, g1_row, b1_pc)
            k_chunk(2)
            v_chunk(2)
            ln_stats(3, TQ)
            ln_apply(3, x_sb, h8, TQ, g1_row, b1_pc)
            k_chunk(3)
            v_chunk(3)

        free_x()
        w8pool.close()
        free_h8()

        zT, _f_zT = tc.tile([P, CB, TQ], F32R, name="zT")
        oAz, _f_oAz = tc.tile([P, CB, TQ], F32R, name="oAz")
        h2T, _f_h2 = tc.tile([P, CB, TQ], BF16, name="h2T")
        h2q8, _f_h2q = tc.tile([P, CB, TQ], FP8, name="h2q8")
        h2r8, _f_h2r = tc.tile([P, CB, TQ], FP8, name="h2r8")
        aT8, _f_a8 = tc.tile([P, FB, TQ], FP8, name="aT8")
        aTr8, _f_ar8 = tc.tile([P, FB, TQ], FP8, name="aTr8")

        # prefetch wo and first FFN1 weights during attention
        # (w1c opens first: pools are a LIFO stack and wo_p closes earlier)
        w1pool = contextlib.ExitStack()
        w1c = w1pool.enter_context(tc.tile_pool(name="w1c", bufs=2))
        wo_pool = contextlib.ExitStack()
        wo_sb = wo_pool.enter_context(tc.tile_pool(name="wo_p", bufs=1)).tile(
            [P, CB, C], FP8, name="wo_sb")
        nc.sync.dma_start(out=wo_sb, in_=wo8[:, :].rearrange("(k p) n -> p k n", p=P))

        # ================= attention (per head pair) =================
        with contextlib.ExitStack() as p3:
            sc_ps_pool = p3.enter_context(tc.tile_pool(name="sc_ps", bufs=2, space="PSUM"))
            pair_ps_pool = p3.enter_context(tc.tile_pool(name="pair_ps", bufs=2, space="PSUM"))
            bc_pool = p3.enter_context(tc.tile_pool(name="bc", bufs=2))
            probs_pool = p3.enter_context(tc.tile_pool(name="probs", bufs=2))
            rec_pool = p3.enter_context(tc.tile_pool(name="rec", bufs=2))

            # key-block pairs (2-bank batched exp; boundary overcompute is
            # masked/unread)
            SBG = [[0, 1], [2, 3], [4, 5], [6, 7], [8, 9], [10, 11],
                   [12, 13], [14, 15]]
            for pair in range(H // 2):
                probs8 = probs_pool.tile([P, 2, NSB, TQ], FP8, name="probs8")
                ps_h = [pair_ps_pool.tile([HD + 1, TQ], F32, name=f"ps_h{u}")
                        for u in range(2)]
                for grp in SBG:
                    q0 = QL[grp[0]]
                    n = len(grp)
                    for u in range(2):
                        prow = slice(u * HD, (u + 1) * HD)
                        ps3 = sc_ps_pool.tile([P, 2, TQ], F32, name="ps_s")
                        for i, sb in enumerate(grp):
                            qi = QL[sb]
                            nc.tensor.matmul(
                                ps3[:, i, qi:TQ],
                                kT[prow, pair, sb * P : (sb + 1) * P],
                                qT[prow, pair, qi:TQ],
                                start=True, stop=True)
                        nc.scalar.activation(
                            probs8[:, u, grp[0] : grp[0] + n, q0:TQ],
                            ps3[:, 0:n, q0:TQ],
                            mybir.ActivationFunctionType.Exp,
                            scale=SCALE2, bias=log32_t)
                        for sb in grp:
                            qb = QL[sb]
                            eng = nc.gpsimd if sb % 3 == 2 else nc.vector
                            eng.tensor_mul(
                                probs8[:, u, sb, qb : qb + P],
                                probs8[:, u, sb, qb : qb + P],
                                mask_sb[:, sb, :])
                for sb in range(NSB):
                    first, last = (sb == 0), (sb == NSB - 1)
                    for u in range(2):
                        nc.tensor.matmul(
                            ps_h[u][:, QL[sb]:TQ],
                            v_aug[:, sb, 2 * pair + u, :],
                            probs8[:, u, sb, QL[sb]:TQ],
                            start=first, stop=last)
                rec_pair = rec_pool.tile([1, 2, TQ], BF16, name="rec_pair")
                for u in range(2):
                    with nc.allow_low_precision(reason="softmax denom"):
                        nc.vector.reciprocal(rec_pair[:, u, :],
                                             ps_h[u][HD : HD + 1, :])
                bc_sb = bc_pool.tile([HD, 2, TQ], BF16, name="bc_sb")
                nc.gpsimd.partition_broadcast(bc_sb, rec_pair)
                for u in range(2):
                    nc.vector.tensor_mul(attn8u[u][:, pair, :],
                                         ps_h[u][0:HD, :], bc_sb[:, u, :])
                # stack this pair onto 128 partitions early (SBUF->SBUF DMA)
                nc.sync.dma_start(out=attn128[0:HD, pair, :],
                                  in_=attn8u[0][:, pair, :])
                nc.sync.dma_start(out=attn128[HD:P, pair, :],
                                  in_=attn8u[1][:, pair, :])

        # ========== wo (fp8 DR) + residual -> zT, LN2 fused into the loop ==========
        with contextlib.ExitStack() as p4:
            lns = p4.enter_context(tc.tile_pool(name="l2_sq", bufs=3))
            lnr = p4.enter_context(tc.tile_pool(name="l2_rows", bufs=1))
            tmp_p = p4.enter_context(tc.tile_pool(name="l2_tmp", bufs=2))
            ab_p = p4.enter_context(tc.tile_pool(name="ab", bufs=2))
            l2_scope = contextlib.ExitStack()
            stat_ps = l2_scope.enter_context(tc.tile_pool(name="l2_stat", bufs=1, space="PSUM"))
            ap_ps = l2_scope.enter_context(tc.tile_pool(name="l2_ap", bufs=1, space="PSUM"))
            wo_scope = contextlib.ExitStack()
            ops = wo_scope.enter_context(tc.tile_pool(name="wo_ps", bufs=2, space="PSUM"))

            m_ps = stat_ps.tile([1, TQ], F32, name="m_ps")
            s_ps = stat_ps.tile([1, TQ], F32, name="s_ps")

            def wo_mb(mb):
                ps = ops.tile([P, 2, 256], F32, name="ps_z")
                for f in range(2):
                    fsl = slice(f * 256, (f + 1) * 256)
                    for kp in range(KP):
                        ksl = slice(2 * kp, 2 * kp + 2)
                        nc.tensor.matmul(
                            ps[:, f, :], wo_sb[:, ksl, mb * P : (mb + 1) * P],
                            attn128[:, ksl, fsl],
                            start=(kp == 0), stop=(kp == KP - 1), perf_mode=DR)
                nc.vector.scalar_tensor_tensor(
                    out=zT[:, mb, :], in0=ps.rearrange("p f n -> p (f n)"),
                    scalar=1.0 / (WS * WS), in1=xpbo[:, mb, :],
                    op0=MUL, op1=ADD)

            def l2_mean(cb):
                nc.tensor.matmul(m_ps, ones_fr, zT[:, cb, :],
                                 start=(cb == 0), stop=(cb == CB - 1),
                                 skip_group_check=True)

            def l2_sq(cb):
                sq = lns.tile([P, TQ], F32R, name="sq")
                nc.scalar.activation(sq, zT[:, cb, :],
                                     mybir.ActivationFunctionType.Square)
                nc.tensor.matmul(s_ps, ones_fr, sq,
                                 start=(cb == 0), stop=(cb == CB - 1),
                                 skip_group_check=True)

            for mb in range(CB):
                wo_mb(mb)
                if mb >= 1:
                    l2_mean(mb - 1)
                if mb >= 2:
                    l2_sq(mb - 2)
            l2_mean(CB - 1)
            l2_sq(CB - 2)
            l2_sq(CB - 1)
            wo_scope.close()
            def ffn2_pass(half, src_acc, dst_write):
                k0 = half * (FB // 2)
                for mb in range(CB):
                    w2_c = w2c.tile([P, FB // 2, P], FP8, name="w2_c", bufs=2)
                    nc.sync.dma_start(
                        out=w2_c,
                        in_=w2[:, :].rearrange("(k p) n -> p k n", p=P)[
                            :, k0 : k0 + FB // 2, mb * P : (mb + 1) * P])
                    ps = fps2.tile([P, 2, 256], F32, name="ps_o")
                    for t0 in range(2):
                        tsl = slice(t0 * 256, (t0 + 1) * 256)
                        for ti, src8 in enumerate((aT8, aTr8)):
                            for kp in range(FB // 4):
                                ksl = slice(2 * kp, 2 * kp + 2)
                                nc.tensor.matmul(
                                    ps[:, t0, :], w2_c[:, ksl, :],
                                    src8[:, k0 + 2 * kp : k0 + 2 * kp + 2, tsl],
                                    start=(ti == 0 and kp == 0),
                                    stop=(ti == 1 and kp == FB // 4 - 1),
                                    perf_mode=DR)
                    dst_write(mb, ps)

            m_sb = lnr.tile([1, TQ], F32, name="m_sb")
            nc.scalar.mul(m_sb, m_ps, 1.0 / C)
            var = lnr.tile([1, TQ], F32, name="var")
            nc.scalar.mul(var, s_ps, 1.0 / C)
            msq = lnr.tile([1, TQ], F32, name="msq")
            nc.vector.tensor_mul(msq, m_sb, m_sb)
            nc.vector.tensor_sub(var, var, msq)
            nc.scalar.activation(var, var, mybir.ActivationFunctionType.Sqrt,
                                 bias=eps_t)
            rstd = lnr.tile([1, TQ], BF16, name="rstd")
            with nc.allow_low_precision(reason="f32r rstd"):
                nc.vector.reciprocal(rstd, var)
            nm = lnr.tile([1, TQ], BF16, name="nm")
            nc.vector.tensor_mul(nm, m_sb, rstd)
            nc.scalar.mul(nm, nm, -1.0)

            g_row = g_rows[:, 1, :]
            sc_ps = ap_ps.tile([P, TQ], F32, name="sc_ps")
            bi_ps = ap_ps.tile([P, TQ], F32, name="bi_ps")
            for cb in range(CB):
                csl = slice(cb * P, (cb + 1) * P)
                nc.tensor.matmul(sc_ps, g_row[:, csl], rstd, start=True, stop=True)
                nc.tensor.matmul(bi_ps, g_row[:, csl], nm, start=True, stop=True)
                tmp = tmp_p.tile([P, TQ], F32R, name="tmp")
                nc.vector.tensor_mul(tmp, zT[:, cb, :], sc_ps)
                nc.vector.scalar_tensor_tensor(
                    out=h2T[:, cb, :], in0=tmp,
                    scalar=b2_pc[:, cb : cb + 1], in1=bi_ps,
                    op0=ADD, op1=ADD)
                nc.scalar.mul(h2q8[:, cb, :], h2T[:, cb, :], 1.0)
                nc.vector.tensor_sub(h2r8[:, cb, :], h2T[:, cb, :],
                                     h2q8[:, cb, :])
                # fold bf2 into zT now that LN2 is done with it
                nc.gpsimd.tensor_scalar_add(zT[:, cb, :], zT[:, cb, :],
                                            bf2_pc[:, cb : cb + 1])

            l2_scope.close()
            fps = p4.enter_context(tc.tile_pool(name="ffn_ps", bufs=4, space="PSUM"))
            fps2 = p4.enter_context(tc.tile_pool(name="ffn2_ps", bufs=4, space="PSUM"))
            w2c = p4.enter_context(tc.tile_pool(name="w2c", bufs=2))
            outp = p4.enter_context(tc.tile_pool(name="outp", bufs=2))

            # ---------------- FFN1: (h2q8 + h2r8) @ w1 (fp8 DR) ----------------
            def ffn1_wg(wg):
                wtile = w1c.tile([P, CB, 4 * P], FP8, name="w1_c", bufs=2)
                nc.sync.dma_start(
                    out=wtile,
                    in_=w1[:, :].rearrange("(k p) n -> p k n", p=P)[
                        :, :, wg * 4 * P : (wg + 1) * 4 * P])
                for fi in range(4):
                    fb = wg * 4 + fi
                    ps = fps.tile([P, 2, 256], F32, name="ps_a")
                    for t0 in range(2):
                        tsl = slice(t0 * 256, (t0 + 1) * 256)
                        for ti, src8 in enumerate((h2q8, h2r8)):
                            for kp in range(KP):
                                ksl = slice(2 * kp, 2 * kp + 2)
                                nc.tensor.matmul(
                                    ps[:, t0, :],
                                    wtile[:, ksl, fi * P : (fi + 1) * P],
                                    src8[:, ksl, tsl],
                                    start=(ti == 0 and kp == 0),
                                    stop=(ti == 1 and kp == KP - 1),
                                    perf_mode=DR)
                    aTb = ab_p.tile([P, TQ], BF16, name="aTb")
                    nc.scalar.activation(aTb, ps.rearrange("p f n -> p (f n)"),
                                         mybir.ActivationFunctionType.Relu,
                                         scale=1.0 / WS,
                                         bias=bf1_pc[:, fb : fb + 1])
                    nc.vector.tensor_copy(aT8[:, fb, :], aTb)
                    nc.vector.tensor_sub(aTr8[:, fb, :], aTb, aT8[:, fb, :])

            def wA(mb, ps):
                nc.vector.scalar_tensor_tensor(
                    out=oAz[:, mb, :], in0=ps.rearrange("p f n -> p (f n)"),
                    scalar=1.0 / WS2, in1=zT[:, mb, :], op0=MUL, op1=ADD)

            def wB(mb, ps):
                o_sb = outp.tile([P, TQ], F32, name="o_sb")
                nc.vector.scalar_tensor_tensor(
                    out=o_sb, in0=ps.rearrange("p f n -> p (f n)"),
                    scalar=1.0 / WS2, in1=oAz[:, mb, :], op0=MUL, op1=ADD)
                nc.sync.dma_start(
                    out=outT[:, :].rearrange("(k p) t -> p k t", p=P)[:, mb, :],
                    in_=o_sb)

            for wg in range(4):
                ffn1_wg(wg)
            ffn2_pass(0, None, wA)
            for wg in range(4, 8):
                ffn1_wg(wg)
            ffn2_pass(1, None, wB)
        wo_pool.close()
        w1pool.close()

        # release singleton tiles in LIFO order
        _f_ar8()
        _f_a8()
        _f_h2r()
        _f_h2q()
        _f_h2()
        _f_oAz()
        _f_zT()
        _f_a128()
        _att[1][1]()
        _att[0][1]()
        _f_v()
        _f_kT()
        _f_qT()
        _f_xpbo()
    nc.compile()
    return nc


_CACHE = {}


def _get_built():
    if "nc" not in _CACHE:
        _CACHE["nc"] = build_kernel()
    return _CACHE["nc"]


def _qidx(j):
    return np.concatenate([np.arange((4 * i + j) * P, (4 * i + j + 1) * P)
                           for i in range(4)])


def _perm_times(j):
    own = [4 * i + j for i in range(4)]
    other = sorted(set(range(NSB)) - set(own))
    return own + other


def _build_in_maps(x, wq, wk, wv, wo, bo, g1, b1, g2, b2, w1, bf1, w2, bf2):
    f = np.float32
    bf = ml_dtypes.bfloat16
    f8 = ml_dtypes.float8_e4m3
    x = np.asarray(x, f)
    wq_m = (np.asarray(wq, f).transpose(1, 0, 2).reshape(C, C) * WS).astype(f8)
    wk_m = (np.asarray(wk, f).transpose(1, 0, 2).reshape(C, C) * WS).astype(f8)
    wv_m = (np.asarray(wv, f).transpose(1, 0, 2).reshape(C, C) * WS).astype(f8)
    wo_m = (np.asarray(wo, f) * WS).astype(f8)
    w1_m = (np.asarray(w1, f) * WS).astype(f8)
    w2_m = (np.asarray(w2, f) * WS2).astype(f8)
    gb_m = np.ascontiguousarray(np.stack([np.asarray(a, f) for a in
                                          (g1, b1, g2, b2, bo, bf2)]))
    bf1_m = np.ascontiguousarray(np.asarray(bf1, f))

    in_maps = []
    for c in range(8):
        b, j = divmod(c, 4)
        ptimes = _perm_times(j)
        tok = np.concatenate([np.arange(t * P, (t + 1) * P) for t in ptimes])
        xT = np.ascontiguousarray(x[b].T[:, tok]).astype(bf)
        # boundary mask per key block kb: q block QL[kb]//P vs key time
        pp = np.arange(P)[:, None]
        cc = np.arange(P)[None, :]
        maskT = np.empty((P, NSB, P), f)
        for kb in range(NSB):
            qt = 4 * (QL[kb] // P) + j
            kt = ptimes[kb]
            maskT[:, kb, :] = ((qt - kt) * P + cc >= pp).astype(f)
        in_maps.append({
            "xT": xT, "maskT": maskT.astype(f8),
            "wq8": wq_m, "wk8": wk_m, "wv8": wv_m, "wo8": wo_m,
            "w1": w1_m, "w2": w2_m, "gb": gb_m, "bf1": bf1_m,
            "ones_in": np.ones((1, P), np.float32),
        })
    return in_maps


def _gather(results):
    out = np.empty((B, T, C), np.float32)
    for c in range(8):
        b, j = divmod(c, 4)
        out[b, _qidx(j)] = results[c]["outT"].T
    return out


def kernel(**inputs):
    in_maps = _build_in_maps(**inputs)
    nc = _get_built()
    res = run_bass_kernel_spmd(nc, in_maps, core_ids=list(range(8)))
    return _gather(res.results)


def run_traced(**inputs):
    in_maps = _build_in_maps(**inputs)
    nc = _get_built()
    return run_bass_kernel_spmd(nc, in_maps, core_ids=list(range(8)), trace=True)
